# revision 1
# baseline (speedup 1.0000x reference)
"""Trainium2 Bass kernel for Chronos2Attention (B=4, S=2048, D=1024, H=16, Dh=64).

Sharding: 8 cores = 4 batches x 2 head-groups. Core c handles batch c//2 and
heads 8*(c%2) .. 8*(c%2)+7 (wq/wk/wv column-sharded, wo row-sharded); host sums
the two partial [S, D] outputs per batch at gather time.

Per-core pipeline (v3 — K/V-first prefix, exp-paced attention, deep interleave):
  Prefix: K and V projected forward per 128-seq chunk (f32r, K=1024 accum),
  K roped on DVE in natural layout and PE-transposed into kT [dims, S]; V
  evicted to bf16 v1 [keys, 8*(64+ones)] on Pool. Q for block 0 projected at
  the tail of the prefix.
  Attention per (j = 512-query block, d = head pair): scores^T on PE f32r,
  exp on ACT one m ahead (bf16 out, ~1.0us per [128,1024] tile — ACT paces
  phase B), AV FLIPPED: out [128 queries, 65 dims] with lhsT = exp tile and
  rhs = bf16 v slice, 65 PE rows per accumulation pass instead of 512 (bf16
  keeps 1 cycle/row below the 256-row f32r threshold). Ones column makes col
  64 the softmax denominator; normalization = per-partition reciprocal (DVE)
  + tensor_scalar multiply (Pool) into bf16 att tiles.
  Interleaved into the exp-paced sweeps: Q proj/rope/transpose for j+1,
  attn^T bf16 PE transposes and the bf16 Wo projection for j-1/j, keeping the
  PE dense while ACT streams exps.

Engine budget per core (cost model): ACT ~266us exp (phase-B pacer),
PE ~295us total, DVE ~90us, Pool ~75us.
"""

import numpy as np

import concourse.bacc as bacc
import concourse.mybir as mybir
import concourse.tile as tile
from concourse import bass_utils

# Problem shapes (hardcoded per spec)
B = 4
S = 2048
D = 1024
H = 16
DH = 64
ROPE_THETA = 10000.0
NCORES = 8
HC = H // 2  # heads per core
MC = HC * DH  # 512, per-core projection width

SM = S // 128  # 16 seq chunks
KD = D // 128  # 8 contraction chunks for projections
MD = MC // 128  # 4 head-dim chunks per core
JBLK = 512  # query-block size for attention
NJ = S // JBLK  # 4
TG = 4  # transpose group size

F32 = mybir.dt.float32
MF = mybir.dt.float32r  # projections/scores
BF = mybir.dt.bfloat16  # exp/V/attn/Wo path


def build_nc():
    nc = bacc.Bacc("TRN2", target_bir_lowering=False, debug=False, num_devices=1)

    hT = nc.dram_tensor("hT", [SM, 128, D], MF, kind="ExternalInput").ap()
    wq = nc.dram_tensor("wq", [128, KD * MC], MF, kind="ExternalInput").ap()
    wk = nc.dram_tensor("wk", [128, KD * MC], MF, kind="ExternalInput").ap()
    wv = nc.dram_tensor("wv", [128, KD * MC], MF, kind="ExternalInput").ap()
    wo = nc.dram_tensor("wo", [128, MD * D], BF, kind="ExternalInput").ap()
    cosq = nc.dram_tensor("cosq", [128, SM * DH], F32, kind="ExternalInput").ap()
    sinq = nc.dram_tensor("sinq", [128, SM * DH], F32, kind="ExternalInput").ap()
    ident = nc.dram_tensor("ident", [128, 128], MF, kind="ExternalInput").ap()
    identb = nc.dram_tensor("identb", [128, 128], BF, kind="ExternalInput").ap()
    out = nc.dram_tensor("out", [S, D], F32, kind="ExternalOutput").ap()

    with tile.TileContext(nc) as tc:
        _build_body(nc, tc, hT, wq, wk, wv, wo, cosq, sinq, ident, identb, out)
    nc.compile()
    return nc


def _build_body(nc, tc, hT, wq, wk, wv, wo, cosq, sinq, ident, identb, out):
    from contextlib import ExitStack

    Exp = mybir.ActivationFunctionType.Exp

    with ExitStack() as ctx:
        # ---- persistent tiles ----
        persist = ctx.enter_context(tc.tile_pool(name="persist", bufs=1))
        kt = [persist.tile([128, S], MF, tag=f"kt{d}", name=f"kt{d}") for d in range(MD)]
        # v1[p, m*520 + h*65 + e]: e<64 -> V dims, e==64 -> ones (softmax denom)
        v1 = persist.tile([128, SM * (HC * 65)], BF, tag="v1", name="v1")
        wo_t = persist.tile([128, MD * D], BF, tag="wo", name="wo_t")
        cos_t = persist.tile([128, SM * DH], F32, tag="cos", name="cos_t")
        sin_t = persist.tile([128, SM * DH], F32, tag="sin", name="sin_t")
        ident_t = persist.tile([128, 128], MF, tag="ident", name="ident_t")
        identb_t = persist.tile([128, 128], BF, tag="identb", name="identb_t")

        wqpool = ctx.enter_context(tc.tile_pool(name="wqpool", bufs=1))
        wq_t = wqpool.tile([128, KD * MC], MF, tag="w_q", name="w_q")
        wvpool = ctx.enter_context(tc.tile_pool(name="wvpool", bufs=1))
        wv_t = wvpool.tile([128, KD * MC], MF, tag="w_v", name="w_v")
        vhpool = ctx.enter_context(tc.tile_pool(name="vh", bufs=3))
        tmp = ctx.enter_context(tc.tile_pool(name="ropetmp", bufs=2))
        qpool = ctx.enter_context(tc.tile_pool(name="qtj", bufs=2))
        hqpool = ctx.enter_context(tc.tile_pool(name="hq", bufs=4))
        rqpool = ctx.enter_context(tc.tile_pool(name="rq", bufs=5))

        # PSUM: sc ring 2x[128,1024] (4 banks) + misc 1x[128,1024] (2) +
        # av 2x[128,260] (<=2) = 8 banks
        scp = ctx.enter_context(tc.tile_pool(name="scp", bufs=2, space="PSUM"))
        miscp = ctx.enter_context(tc.tile_pool(name="miscp", bufs=1, space="PSUM"))
        avp = ctx.enter_context(tc.tile_pool(name="avp", bufs=2, space="PSUM"))

        def rope(ps, r, m):
            """Natural-layout RoPE: ps [128 seq, HC*DH dims] -> r (f32r)."""
            cos_m = cos_t[:, None, m * DH : (m + 1) * DH]
            sin_m = sin_t[:, m * DH : (m + 1) * DH]
            tc_ = tmp.tile([128, MC], F32, tag="tc", name="tc_")
            ts_ = tmp.tile([128, MC], F32, tag="ts", name="ts_")
            p3 = ps.rearrange("p (h e) -> p h e", h=HC)
            t3 = ts_[:].rearrange("p (h e) -> p h e", h=HC)
            nc.vector.tensor_mul(
                tc_[:].rearrange("p (h e) -> p h e", h=HC),
                p3,
                cos_m.broadcast_to([128, HC, DH]),
            )
            nc.vector.tensor_mul(
                t3[:, :, 0:32],
                p3[:, :, 32:64],
                sin_m[:, None, 0:32].broadcast_to([128, HC, 32]),
            )
            nc.vector.tensor_mul(
                t3[:, :, 32:64],
                p3[:, :, 0:32],
                sin_m[:, None, 32:64].broadcast_to([128, HC, 32]),
            )
            nc.vector.tensor_add(r[:], tc_[:], ts_[:])

        # ---- Q machinery (used for j=0 in the prefix, then interleaved) ----
        qstate = {}

        def qproj_piece(jq, k):
            """Project+rope query chunk k (seq chunk jq*4+k) of block jq."""
            st = qstate.setdefault(jq, {"h": [None] * 4, "rot": [None] * 4})
            if k == 0:
                for kk2 in range(4):
                    t = hqpool.tile([128, D], MF, tag="hq", name="hq")
                    nc.sync.dma_start(out=t[:], in_=hT[jq * 4 + kk2])
                    st["h"][kk2] = t
            h_m = st["h"][k]
            ps = miscp.tile([128, MC], F32, tag="misc", name="qps")
            for kk in range(KD):
                nc.tensor.matmul(
                    ps[:],
                    h_m[:, kk * 128 : (kk + 1) * 128],
                    wq_t[:, kk * MC : (kk + 1) * MC],
                    start=(kk == 0),
                    stop=(kk == KD - 1),
                )
            r = rqpool.tile([128, MC], MF, tag="rq", name="rq")
            rope(ps[:], r, jq * 4 + k)
            st["rot"][k] = r

        def qtrans_piece(jq, on_act=False):
            """Transpose block jq's roped Q into qtj [dims, 512]."""
            st = qstate[jq]
            qtj = [
                qpool.tile([128, JBLK], MF, tag=f"qt{d}", name=f"qt{d}")
                for d in range(MD)
            ]
            for dpair in range(2):
                tps = miscp.tile([128, 1024], MF, tag="misc", name="qtps")
                for half in range(2):
                    d = dpair * 2 + half
                    for mm in range(4):
                        nc.tensor.transpose(
                            tps[:, half * 512 + mm * 128 : half * 512 + (mm + 1) * 128],
                            st["rot"][mm][:, d * 128 : (d + 1) * 128],
                            ident_t[:],
                        )
                for half in range(2):
                    d = dpair * 2 + half
                    src = tps[:, half * 512 : (half + 1) * 512]
                    if on_act:
                        nc.scalar.copy(qtj[d][:], src)
                    else:
                        nc.vector.tensor_copy(qtj[d][:], src)
            st["qtj"] = qtj

        # ---- prefix: front DMAs ordered for earliest first matmul ----
        with ExitStack() as actx:
            wkvp = actx.enter_context(tc.tile_pool(name="wkv", bufs=1))
            wk_t = wkvp.tile([128, KD * MC], MF, tag="w_k", name="w_k")
            hpool = actx.enter_context(tc.tile_pool(name="hpre", bufs=5))
            rkpool = actx.enter_context(tc.tile_pool(name="rk", bufs=5))

            h_tiles = [None] * SM

            def dma_h(m):
                h_tiles[m] = hpool.tile([128, D], MF, tag="h", name="h_m")
                nc.sync.dma_start(out=h_tiles[m][:], in_=hT[m])

            # dep-free PE warmup from memset tiles: finishes the p-state ramp
            # (~3us of continuous execution) while the first DMAs stream
            wu = persist.tile([128, 512], F32, tag="wu", name="wu")
            nc.vector.memset(wu[:], 0.0)
            wups = miscp.tile([128, 512], F32, tag="misc", name="wups")
            for i in range(4):
                nc.tensor.matmul(
                    wups[:], wu[:, 0:128], wu[:], start=(i == 0), stop=(i == 3)
                )

            dma_h(0)
            for kk in range(KD):
                nc.sync.dma_start(
                    out=wk_t[:, kk * MC : (kk + 1) * MC],
                    in_=wk[:, kk * MC : (kk + 1) * MC],
                )
                if kk == 0:
                    nc.sync.dma_start(out=cos_t[:], in_=cosq)
                    nc.sync.dma_start(out=sin_t[:], in_=sinq)
            dma_h(1)
            dma_h(2)
            nc.sync.dma_start(out=ident_t[:], in_=ident)
            dma_h(3)
            dma_h(4)
            nc.sync.dma_start(out=identb_t[:], in_=identb)
            for kk in range(KD):
                nc.sync.dma_start(
                    out=wq_t[:, kk * MC : (kk + 1) * MC],
                    in_=wq[:, kk * MC : (kk + 1) * MC],
                )
            for kk in range(KD):
                nc.sync.dma_start(
                    out=wv_t[:, kk * MC : (kk + 1) * MC],
                    in_=wv[:, kk * MC : (kk + 1) * MC],
                )
            nc.sync.dma_start(out=wo_t[:], in_=wo)

            # ones columns of v1; warm the ACT exp table early
            oneb = persist.tile([128, 1], BF, tag="oneb", name="oneb")
            nc.gpsimd.memset(oneb[:], 1.0)
            nc.vector.tensor_copy(
                v1[:].rearrange("p (m h e) -> p m h e", m=SM, h=HC)[:, :, :, 64:65],
                oneb[:, None, None, 0:1].broadcast_to([128, SM, HC, 1]),
            )
            warm = persist.tile([1, 16], F32, tag="warm", name="warm")
            nc.vector.memset(warm[:], 0.0)
            nc.scalar.activation(warm[:], warm[:], Exp)

            # ---- prefix: K (rope+transpose) and V per seq chunk ----
            rot_k = [None] * TG
            for m in range(SM):
                if m + 5 < SM:
                    dma_h(m + 5)
                h_m = h_tiles[m]
                ps = scp.tile([128, MC], F32, tag="sc", name="kps")
                for kk in range(KD):
                    nc.tensor.matmul(
                        ps[:],
                        h_m[:, kk * 128 : (kk + 1) * 128],
                        wk_t[:, kk * MC : (kk + 1) * MC],
                        start=(kk == 0),
                        stop=(kk == KD - 1),
                    )
                r = rkpool.tile([128, MC], MF, tag="rk", name="rk")
                rope(ps[:], r, m)
                rot_k[m % TG] = r

                if m % TG == TG - 1:
                    m0 = m - (TG - 1)
                    for dpair in range(2):
                        tps = scp.tile([128, 1024], MF, tag="sc", name="ktps")
                        for half in range(2):
                            d = dpair * 2 + half
                            for mm in range(TG):
                                nc.tensor.transpose(
                                    tps[:, half * 512 + mm * 128 : half * 512 + (mm + 1) * 128],
                                    rot_k[mm][:, d * 128 : (d + 1) * 128],
                                    ident_t[:],
                                )
                        for half in range(2):
                            d = dpair * 2 + half
                            nc.scalar.copy(
                                kt[d][:, m0 * 128 : m0 * 128 + TG * 128],
                                tps[:, half * 512 : (half + 1) * 512],
                            )
                # Q block 0 at the prefix tail, interleaved with last chunks
                if m == 11:
                    qproj_piece(0, 0)
                elif m == 12:
                    qproj_piece(0, 1)
                elif m == 13:
                    qproj_piece(0, 2)
                elif m == 14:
                    qproj_piece(0, 3)
                elif m == 15:
                    qtrans_piece(0)

        # ---- deferred V projection: key chunk m, run inside j0's d=0 sweep
        vh_tiles = {}

        def v_dma(m):
            t = vhpool.tile([128, D], MF, tag="vh", name="vh")
            nc.sync.dma_start(out=t[:], in_=hT[m])
            vh_tiles[m] = t

        def v_piece(m):
            if m + 2 < SM:
                v_dma(m + 2)
            h_m = vh_tiles.pop(m)
            vp = miscp.tile([128, MC], F32, tag="misc", name="vps")
            for kk in range(KD):
                nc.tensor.matmul(
                    vp[:],
                    h_m[:, kk * 128 : (kk + 1) * 128],
                    wv_t[:, kk * MC : (kk + 1) * MC],
                    start=(kk == 0),
                    stop=(kk == KD - 1),
                )
            dst = v1[:, m * (HC * 65) : (m + 1) * (HC * 65)].rearrange(
                "p (h e) -> p h e", h=HC
            )[:, :, 0:64]
            nc.vector.tensor_copy(dst, vp[:].rearrange("p (h e) -> p h e", h=HC))

        v_dma(0)
        v_dma(1)

        # ---- phase B pools (allocated after prefix pools are freed) ----
        expp = ctx.enter_context(tc.tile_pool(name="expp", bufs=3))
        rcpp = ctx.enter_context(tc.tile_pool(name="rcpp", bufs=8))
        attp = ctx.enter_context(tc.tile_pool(name="attp", bufs=2))
        attTp = ctx.enter_context(tc.tile_pool(name="attTp", bufs=2))
        outp = ctx.enter_context(tc.tile_pool(name="outp", bufs=2))

        att_state = {}

        def attT_piece(jw, pair, pool=None):
            """Transpose att columns of head-pairs d=2*pair,2*pair+1."""
            att_j, attT_j = att_state[jw]
            pool = pool or miscp
            tag = "misc" if pool is miscp else "sc"
            tps = pool.tile([128, 1024], BF, tag=tag, name="atps")
            for half in range(2):
                kk = pair * 2 + half
                for qc in range(4):
                    nc.tensor.transpose(
                        tps[:, half * 512 + qc * 128 : half * 512 + (qc + 1) * 128],
                        att_j[qc][:, kk * 128 : (kk + 1) * 128],
                        identb_t[:],
                    )
            for half in range(2):
                kk = pair * 2 + half
                t = attTp.tile([128, JBLK], BF, tag=f"attT{kk}", name="attT")
                nc.vector.tensor_copy(t[:], tps[:, half * 512 : (half + 1) * 512])
                attT_j[kk] = t

        def wo_piece(jw, qc, pool=None):
            attT_j = att_state[jw][1]
            pool = pool or miscp
            tag = "misc" if pool is miscp else "sc"
            wops = pool.tile([128, 1024], F32, tag=tag, name="wops")
            for nb in range(2):
                for kk in range(MD):
                    nc.tensor.matmul(
                        wops[:, nb * 512 : (nb + 1) * 512],
                        attT_j[kk][:, qc * 128 : (qc + 1) * 128],
                        wo_t[:, kk * D + nb * 512 : kk * D + nb * 512 + 512],
                        start=(kk == 0),
                        stop=(kk == MD - 1),
                    )
            ot = outp.tile([128, D], F32, tag="ot", name="ot")
            nc.vector.tensor_copy(ot[:], wops[:])
            mrow = jw * JBLK + qc * 128
            nc.sync.dma_start(out=out[mrow : mrow + 128, :], in_=ot[:])

        # ---- phase B: attention per query block ----
        for j in range(NJ):
            qtj = qstate[j]["qtj"]
            att_j = [
                attp.tile([128, JBLK], BF, tag=f"att{qc}", name=f"att{qc}")
                for qc in range(4)
            ]
            att_state[j] = (att_j, [None] * MD)

            slots = {}

            def add(d, m, fn):
                slots.setdefault((d, m), []).append(fn)

            if j > 0:
                add(0, 3, lambda jw=j - 1: attT_piece(jw, 1))
                add(0, 7, lambda jw=j - 1: wo_piece(jw, 0))
                add(0, 11, lambda jw=j - 1: wo_piece(jw, 1))
                add(1, 3, lambda jw=j - 1: wo_piece(jw, 2))
                add(1, 7, lambda jw=j - 1: wo_piece(jw, 3))
            if j + 1 < NJ:
                add(1, 11, lambda jn=j + 1: qproj_piece(jn, 0))
                add(2, 3, lambda jn=j + 1: qproj_piece(jn, 1))
                add(2, 7, lambda jn=j + 1: qproj_piece(jn, 2))
                add(2, 11, lambda jn=j + 1: qproj_piece(jn, 3))
                add(3, 3, lambda jn=j + 1: qtrans_piece(jn))
            add(3, 7, lambda jw=j: attT_piece(jw, 0))

            for d in range(MD):
                # full-bank tiles (one 2KB zero region each): one accumulation
                # group per bank — start zeroes the bank once (qc==0), the
                # other qc slots accumulate into the pre-zeroed region
                av_h = [
                    avp.tile([128, 512], F32, tag="av", name=f"av{hh}")
                    for hh in range(2)
                ]

                def emit_sc(m):
                    sc = scp.tile([128, 2 * JBLK], F32, tag="sc", name="sc")
                    nc.tensor.matmul(
                        sc[:, 0:JBLK],
                        kt[d][0:64, m * 128 : (m + 1) * 128],
                        qtj[d][0:64, :],
                        start=True,
                        stop=True,
                    )
                    nc.tensor.matmul(
                        sc[:, JBLK : 2 * JBLK],
                        kt[d][64:128, m * 128 : (m + 1) * 128],
                        qtj[d][64:128, :],
                        start=True,
                        stop=True,
                    )
                    return sc

                sc_next = emit_sc(0)
                for m in range(SM):
                    if j == 0 and d == 0:
                        v_piece(m)
                    sc = sc_next
                    if m + 1 < SM:
                        sc_next = emit_sc(m + 1)
                    ex = expp.tile([128, 2 * JBLK], BF, tag="ex", name="ex")
                    nc.scalar.activation(ex[:], sc[:], Exp)
                    for hh in range(2):
                        h = 2 * d + hh
                        vs = m * (HC * 65) + h * 65
                        for qc in range(4):
                            nc.tensor.matmul(
                                av_h[hh][:, qc * 65 : (qc + 1) * 65],
                                ex[:, hh * JBLK + qc * 128 : hh * JBLK + (qc + 1) * 128],
                                v1[:, vs : vs + 65],
                                start=(m == 0 and qc == 0),
                                stop=(m == SM - 1 and qc == 3),
                            )
                    for fn in slots.get((d, m), ()):
                        fn()

                # normalize: col 64 of each slot is the softmax denominator
                for hh in range(2):
                    h2 = 2 * d + hh
                    for qc in range(4):
                        s0 = qc * 65
                        rcp = rcpp.tile([128, 1], F32, tag="rcp", name="rcp")
                        nc.vector.reciprocal(rcp[:], av_h[hh][:, s0 + 64 : s0 + 65])
                        nc.vector.tensor_scalar_mul(
                            att_j[qc][:, h2 * 64 : (h2 + 1) * 64],
                            av_h[hh][:, s0 : s0 + 64],
                            rcp[:],
                        )

        # tail: remaining work for the last block (sc ring is free now,
        # so pieces double-buffer instead of serializing on miscp)
        attT_piece(NJ - 1, 1, pool=scp)
        for qc in range(4):
            wo_piece(NJ - 1, qc, pool=scp)


def _wtile(w, dtype=np.float32):
    """[K*128, N] -> [128, K*N] with tile[p, kk*N+c] = w[128*kk+p, c]."""
    kchunks = w.shape[0] // 128
    return np.ascontiguousarray(
        w.reshape(kchunks, 128, w.shape[1])
        .transpose(1, 0, 2)
        .reshape(128, kchunks * w.shape[1])
        .astype(dtype)
    )


def prep_core_inputs(positions, hidden_states, wq, wk, wv, wo):
    """Host-side sharding/pre-tiling. Returns list of 8 in_maps."""
    import ml_dtypes

    bf16 = ml_dtypes.bfloat16
    pos = np.asarray(positions).astype(np.float32)
    inv_freq = 1.0 / (ROPE_THETA ** (np.arange(0, DH, 2, dtype=np.float32) / DH))
    ang = pos[:, None] * inv_freq[None, :]  # [S, 32]
    cos_half = np.cos(ang).astype(np.float32)
    sin_half = np.sin(ang).astype(np.float32)
    cos_full = np.concatenate([cos_half, cos_half], axis=1)  # [S, 64]
    sin_signed = np.concatenate([-sin_half, sin_half], axis=1)  # [S, 64]
    cos_tiled = np.ascontiguousarray(
        cos_full.reshape(SM, 128, DH).transpose(1, 0, 2).reshape(128, SM * DH)
    )
    sin_tiled = np.ascontiguousarray(
        sin_signed.reshape(SM, 128, DH).transpose(1, 0, 2).reshape(128, SM * DH)
    )
    ident = np.eye(128, dtype=np.float32)
    identb = ident.astype(bf16)

    hs = np.asarray(hidden_states, dtype=np.float32)
    # hT[m, p, kk*128+cc] = hidden[b, 128m+cc, 128kk+p]
    hT_all = []
    for b in range(B):
        t = hs[b].reshape(SM, 128, KD, 128).transpose(0, 3, 2, 1)  # (m, p, kk, cc)
        hT_all.append(np.ascontiguousarray(t.reshape(SM, 128, D)))

    in_maps = []
    for c in range(NCORES):
        b = c // 2
        g = c % 2
        sl = slice(g * MC, (g + 1) * MC)
        in_maps.append(
            {
                "hT": hT_all[b],
                "wq": _wtile(np.asarray(wq, np.float32)[:, sl]),
                "wk": _wtile(np.asarray(wk, np.float32)[:, sl]),
                "wv": _wtile(np.asarray(wv, np.float32)[:, sl]),
                "wo": _wtile(np.asarray(wo, np.float32)[sl, :], dtype=bf16),
                "cosq": cos_tiled,
                "sinq": sin_tiled,
                "ident": ident,
                "identb": identb,
            }
        )
    return in_maps


_NC_CACHE = {}


def get_nc():
    if "nc" not in _NC_CACHE:
        _NC_CACHE["nc"] = build_nc()
    return _NC_CACHE["nc"]


def kernel(positions, hidden_states, wq, wk, wv, wo):
    in_dtype = np.asarray(hidden_states).dtype
    in_maps = prep_core_inputs(positions, hidden_states, wq, wk, wv, wo)
    nc = get_nc()
    res = bass_utils.run_bass_kernel_spmd(nc, in_maps, core_ids=list(range(NCORES)))
    outs = np.empty((B, S, D), dtype=np.float32)
    for b in range(B):
        outs[b] = res.results[2 * b]["out"] + res.results[2 * b + 1]["out"]
    return outs.astype(in_dtype, copy=False)



# revision 55
# speedup vs baseline: 1.1287x; 1.1287x over previous
"""Trainium2 Bass kernel for Chronos2Attention (B=4, S=2048, D=1024, H=16, Dh=64).

Sharding: 8 cores = 4 batches x 2 head-groups. Core c handles batch c//2 and
heads 8*(c%2) .. 8*(c%2)+7 (wq/wk/wv column-sharded, wo row-sharded); host sums
the two partial [S, D] outputs per batch at gather time.

v4 design (vs v3): fully fused production/attention pipeline.
  - All PE-side tensors bf16 (h, weights, roped q/k, kt, v, attn) -> every
    matmul/transpose runs at 1 cyc/row; halved DMA.
  - K/V/Q production streams through a 2-buf [128,512]f32 PSUM "prod" pool
    just-in-time; attention sweeps consume chunks as they appear.
  - Sweeps (j = 512-query block, d = head pair) are SEGMENTED: AV accumulates
    in a single PSUM bank per segment (8 heads-slots x 4qc x 65 cols with the
    ones/denominator column), partial segments evicted/accumulated into SBUF
    f32 "pacc" tiles so only 2 av banks are ever needed while 8+ sweeps are
    in flight against a limited set of produced chunks.
  - Emission is availability-driven: a tiny cost model interleaves production
    passes, sweep iterations (sc->exp->AV), and aux work (norm/attT/wo/q-proj)
    to keep ACT (exp, the ~266us floor) saturated and PE dense.
  - PSUM budget: sc ring 2x[128,1024] (4 banks) + av 2x[128,512] (2) +
    prod 2x[128,512] (2) = 8 banks.
"""

from collections import deque

import numpy as np

import concourse.bacc as bacc
import concourse.mybir as mybir
import concourse.tile as tile
from concourse import bass_utils

# Problem shapes (hardcoded per spec)
B = 4
S = 2048
D = 1024
H = 16
DH = 64
ROPE_THETA = 10000.0
NCORES = 8
HC = H // 2  # heads per core
MC = HC * DH  # 512, per-core projection width

SM = S // 128  # 16 seq chunks
KD = D // 128  # 8 contraction chunks for projections
MD = HC // 2  # 4 head-pair sweeps per j block
JBLK = 512
NJ = S // JBLK  # 4
TG = 2  # k-transpose group size

F32 = mybir.dt.float32
BF = mybir.dt.bfloat16

# emission cost estimates (ns) for the build-time scheduler
EST_PROJ = 1800.0
EST_KPASS = 2800.0  # proj + rope serialization through the prod ring
EST_VPASS = 2200.0
EST_ITER_PE = 700.0
EST_ITER_ACT = 1045.0
EST_SC = 450.0
BACKLOG_LO = 6000.0  # emit sweep iters while ACT backlog below this


def build_nc():
    nc = bacc.Bacc("TRN2", target_bir_lowering=False, debug=False, num_devices=1)

    hT = nc.dram_tensor("hT", [SM, 128, D], BF, kind="ExternalInput").ap()
    wq = nc.dram_tensor("wq", [128, KD * MC], BF, kind="ExternalInput").ap()
    wk = nc.dram_tensor("wk", [128, KD * MC], BF, kind="ExternalInput").ap()
    wv = nc.dram_tensor("wv", [128, KD * MC], BF, kind="ExternalInput").ap()
    wo = nc.dram_tensor("wo", [128, MD * D], BF, kind="ExternalInput").ap()
    cosq = nc.dram_tensor("cosq", [128, SM * DH], F32, kind="ExternalInput").ap()
    sinq = nc.dram_tensor("sinq", [128, SM * DH], F32, kind="ExternalInput").ap()
    identb = nc.dram_tensor("identb", [128, 128], BF, kind="ExternalInput").ap()
    # out[ph]: contribution of head-pairs 2ph..2ph+1 (summed host-side)
    out = nc.dram_tensor("out", [2, S, D], F32, kind="ExternalOutput").ap()

    with tile.TileContext(nc) as tc:
        _build_body(nc, tc, hT, wq, wk, wv, wo, cosq, sinq, identb, out)
    nc.compile()
    return nc


def _build_body(nc, tc, hT, wq, wk, wv, wo, cosq, sinq, identb, out):
    from contextlib import ExitStack

    Exp = mybir.ActivationFunctionType.Exp

    with ExitStack() as ctx:
        # ---- persistent SBUF ----
        persist = ctx.enter_context(tc.tile_pool(name="persist", bufs=1))
        kt = [persist.tile([128, S], BF, tag=f"kt{d}", name=f"kt{d}") for d in range(MD)]
        # v1[p, m*520 + h*65 + e]: e<64 -> V dims, e==64 -> ones (softmax denom)
        v1 = persist.tile([128, SM * (HC * 65)], BF, tag="v1", name="v1")
        wq_t = persist.tile([128, KD * MC], BF, tag="w_q", name="w_q")
        wk_t = persist.tile([128, KD * MC], BF, tag="w_k", name="w_k")
        wv_t = persist.tile([128, KD * MC], BF, tag="w_v", name="w_v")
        wo_t = persist.tile([128, MD * D], BF, tag="wo", name="wo_t")
        cos_t = persist.tile([128, SM * DH], F32, tag="cos", name="cos_t")
        sin_t = persist.tile([128, SM * DH], F32, tag="sin", name="sin_t")
        identb_t = persist.tile([128, 128], BF, tag="identb", name="identb_t")

        # ---- working SBUF pools ----
        hpool = ctx.enter_context(tc.tile_pool(name="hprod", bufs=6))
        hqpool = ctx.enter_context(tc.tile_pool(name="hq", bufs=4))
        rkpool = ctx.enter_context(tc.tile_pool(name="rk", bufs=5))
        kfpool = ctx.enter_context(tc.tile_pool(name="kf", bufs=3))
        rqpool = ctx.enter_context(tc.tile_pool(name="rq", bufs=5))
        qpool = ctx.enter_context(tc.tile_pool(name="qtj", bufs=2))
        tmpp = ctx.enter_context(tc.tile_pool(name="ropetmp", bufs=3))
        expp = ctx.enter_context(tc.tile_pool(name="expp", bufs=10))
        paccp = ctx.enter_context(tc.tile_pool(name="pacc", bufs=1))
        rcpp = ctx.enter_context(tc.tile_pool(name="rcpp", bufs=8))
        attp = ctx.enter_context(tc.tile_pool(name="attp", bufs=2))
        attTp = ctx.enter_context(tc.tile_pool(name="attTp", bufs=2))
        outp = ctx.enter_context(tc.tile_pool(name="outp", bufs=2))

        # ---- PSUM: 4 + 2 + 2 = 8 banks ----
        scp = ctx.enter_context(tc.tile_pool(name="scp", bufs=2, space="PSUM"))
        avp = ctx.enter_context(tc.tile_pool(name="avp", bufs=2, space="PSUM"))
        prodp = ctx.enter_context(tc.tile_pool(name="prodp", bufs=2, space="PSUM"))

        # ---- init: DMAs ordered for earliest first sweep ----
        h_tiles = {}

        def dma_h(m):
            t = hpool.tile([128, D], BF, tag="h", name="h_m")
            nc.sync.dma_start(out=t[:], in_=hT[m])
            h_tiles[m] = t

        half = 4 * MC
        nc.sync.dma_start(out=wq_t[:, 0:half], in_=wq[:, 0:half])
        dma_h(0)
        dma_h(1)
        nc.sync.dma_start(out=wq_t[:, half:], in_=wq[:, half:])
        nc.sync.dma_start(out=wk_t[:, 0:half], in_=wk[:, 0:half])
        nc.sync.dma_start(out=cos_t[:], in_=cosq)
        nc.sync.dma_start(out=sin_t[:], in_=sinq)
        nc.sync.dma_start(out=identb_t[:], in_=identb)
        dma_h(2)
        dma_h(3)
        nc.sync.dma_start(out=wk_t[:, half:], in_=wk[:, half:])
        nc.sync.dma_start(out=wv_t[:, 0:half], in_=wv[:, 0:half])
        nc.sync.dma_start(out=wv_t[:, half:], in_=wv[:, half:])
        dma_h(4)
        dma_h(5)
        nc.sync.dma_start(out=wo_t[:], in_=wo)

        # PE warmup (p-state ramp) on dep-free memset tiles
        wu = persist.tile([128, 512], F32, tag="wu", name="wu")
        nc.vector.memset(wu[:], 0.0)
        wups = prodp.tile([128, 512], F32, tag="prod", name="wups")
        for i in range(4):
            nc.tensor.matmul(
                wups[:], wu[:, 0:128], wu[:], start=(i == 0), stop=(i == 3)
            )

        # ones columns of v1 (Pool memset + DVE broadcast); warm the exp table
        oneb = persist.tile([128, 1], BF, tag="oneb", name="oneb")
        nc.gpsimd.memset(oneb[:], 1.0)
        nc.vector.tensor_copy(
            v1[:].rearrange("p (m h e) -> p m h e", m=SM, h=HC)[:, :, :, 64:65],
            oneb[:, None, None, 0:1].broadcast_to([128, SM, HC, 1]),
        )
        warm = persist.tile([1, 16], F32, tag="warm", name="warm")
        nc.vector.memset(warm[:], 0.0)
        nc.scalar.activation(warm[:], warm[:], Exp)

        # ---- shared helpers ----
        def rope(src, r, m, eng):
            """Natural-layout RoPE: src [128 seq, HC*DH] -> r bf16.
            eng=nc.vector reads PSUM directly (low latency, K path);
            eng=nc.gpsimd needs an SBUF source (Q path, off critical path)."""
            cos_m = cos_t[:, None, m * DH : (m + 1) * DH]
            sin_m = sin_t[:, m * DH : (m + 1) * DH]
            tc_ = tmpp.tile([128, MC], F32, tag="tc", name="tc_")
            ts_ = tmpp.tile([128, MC], F32, tag="ts", name="ts_")
            p3 = src.rearrange("p (h e) -> p h e", h=HC)
            t3 = ts_[:].rearrange("p (h e) -> p h e", h=HC)
            eng.tensor_mul(
                tc_[:].rearrange("p (h e) -> p h e", h=HC),
                p3,
                cos_m.broadcast_to([128, HC, DH]),
            )
            eng.tensor_mul(
                t3[:, :, 0:32],
                p3[:, :, 32:64],
                sin_m[:, None, 0:32].broadcast_to([128, HC, 32]),
            )
            eng.tensor_mul(
                t3[:, :, 32:64],
                p3[:, :, 0:32],
                sin_m[:, None, 32:64].broadcast_to([128, HC, 32]),
            )
            eng.tensor_add(r[:], tc_[:], ts_[:])

        def proj(h_m, w_t):
            ps = prodp.tile([128, MC], F32, tag="prod", name="ps")
            for kk in range(KD):
                nc.tensor.matmul(
                    ps[:],
                    h_m[:, kk * 128 : (kk + 1) * 128],
                    w_t[:, kk * MC : (kk + 1) * MC],
                    start=(kk == 0),
                    stop=(kk == KD - 1),
                )
            return ps

        # ---- production pieces ----
        rot_k = [None] * TG
        qrot = {j: [None] * 4 for j in range(NJ)}
        qtj = {}

        def k_pass(m):
            ps = proj(h_tiles[m], wk_t)
            r = rkpool.tile([128, MC], BF, tag="rk", name="rk")
            rope(ps[:], r, m, nc.vector)
            rot_k[m % TG] = r
            if m % TG == TG - 1:
                m0 = m - (TG - 1)
                W = TG * 128
                for dpair in range(2):
                    tps = prodp.tile([128, 2 * W], BF, tag="prod", name="ktps")
                    for half in range(2):
                        d = dpair * 2 + half
                        for mm in range(TG):
                            nc.tensor.transpose(
                                tps[:, half * W + mm * 128 : half * W + (mm + 1) * 128],
                                rot_k[mm][:, d * 128 : (d + 1) * 128],
                                identb_t[:],
                            )
                    for half in range(2):
                        d = dpair * 2 + half
                        nc.vector.tensor_copy(
                            kt[d][:, m0 * 128 : m0 * 128 + W],
                            tps[:, half * W : (half + 1) * W],
                        )

        def v_pass(m):
            ps = proj(h_tiles[m], wv_t)
            dst = v1[:, m * (HC * 65) : (m + 1) * (HC * 65)].rearrange(
                "p (h e) -> p h e", h=HC
            )[:, :, 0:64]
            nc.vector.tensor_copy(dst, ps[:].rearrange("p (h e) -> p h e", h=HC))
            h_tiles.pop(m)

        def q_pass(j, k):
            m = j * 4 + k
            if j < 2:
                h_m = h_tiles[m]
            else:
                h_m = h_tiles.pop(("q", m))
            ps = proj(h_m, wq_t)
            r = rqpool.tile([128, MC], BF, tag="rq", name="rq")
            rope(ps[:], r, m, nc.vector)
            qrot[j][k] = r

        def q_dma(j, k):
            m = j * 4 + k
            t = hqpool.tile([128, D], BF, tag="hq", name="hq")
            nc.sync.dma_start(out=t[:], in_=hT[m])
            h_tiles[("q", m)] = t

        def q_trans(j):
            tiles = [
                qpool.tile([128, JBLK], BF, tag=f"qt{d}", name=f"qt{d}")
                for d in range(MD)
            ]
            for dpair in range(2):
                tps = prodp.tile([128, 1024], BF, tag="prod", name="qtps")
                for half in range(2):
                    d = dpair * 2 + half
                    for mm in range(4):
                        nc.tensor.transpose(
                            tps[:, half * 512 + mm * 128 : half * 512 + (mm + 1) * 128],
                            qrot[j][mm][:, d * 128 : (d + 1) * 128],
                            identb_t[:],
                        )
                for half in range(2):
                    d = dpair * 2 + half
                    nc.vector.tensor_copy(
                        tiles[d][:], tps[:, half * 512 : (half + 1) * 512]
                    )
            qrot[j] = [None] * 4
            qtj[j] = tiles

        # ---- sweep machinery ----
        # state per sweep (j,d): next_m, av pair or None, pacc tile, pending AV
        att_state = {}  # j -> [att tiles per qc]
        attT_state = {}  # j -> [attT tiles per kk]

        PENDING_DEPTH = 2  # min AV emission lag behind exp (iters)
        CLOSE_LATENCY = 2600.0  # est ns from close emission to av-bank free

        class Sweep:
            def __init__(self, j, d):
                self.j = j
                self.d = d
                self.next_m = 0
                self.av = None  # (av0, av1) psum pair while segment open
                self.seg_start = True  # next pending AV opens a segment
                self.pacc = None
                self.pending = deque()  # (m, ex) awaiting AV emission
                self.done = False

        def emit_pending_av(sw, stop):
            m, ex = sw.pending.popleft()
            if sw.av is None:
                sw.av = (
                    avp.tile([128, 512], F32, tag="av", name="av0"),
                    avp.tile([128, 512], F32, tag="av", name="av1"),
                )
                sw.seg_start = True
            for hh in range(2):
                h2 = 2 * sw.d + hh
                vs = m * (HC * 65) + h2 * 65
                for qc in range(4):
                    nc.tensor.matmul(
                        sw.av[hh][:, qc * 65 : (qc + 1) * 65],
                        ex[:, hh * JBLK + qc * 128 : hh * JBLK + (qc + 1) * 128],
                        v1[:, vs : vs + 65],
                        start=(sw.seg_start and qc == 0),
                        stop=(stop and qc == 3),
                    )
            sw.seg_start = False

        def emit_iter(sw):
            """One m iteration: sc matmuls + exp; AV of previous iter."""
            m = sw.next_m
            sw.next_m += 1
            d, j = sw.d, sw.j
            sc = scp.tile([128, 2 * JBLK], F32, tag="sc", name="sc")
            nc.tensor.matmul(
                sc[:, 0:JBLK],
                kt[d][0:64, m * 128 : (m + 1) * 128],
                qtj[j][d][0:64, :],
                start=True,
                stop=True,
            )
            nc.tensor.matmul(
                sc[:, JBLK : 2 * JBLK],
                kt[d][64:128, m * 128 : (m + 1) * 128],
                qtj[j][d][64:128, :],
                start=True,
                stop=True,
            )
            drain_pending(sw)
            ex = expp.tile([128, 2 * JBLK], BF, tag="ex", name="ex")
            nc.scalar.activation(ex[:], sc[:], Exp)
            sw.pending.append((m, ex))

        av_free_est = [0.0]

        def drain_pending(sw):
            """Emit one deferred AV unless a new segment would have to wait
            for the av pair to drain (let exps run ahead instead). Hard cap:
            an exp's ex-tile allocation WARs against the AV 10 allocations
            back (expp ring) — if AVs lag more than ring-2, the sem graph
            deadlocks (exp waits AV, AV behind a blocked sc, sc waits exp)."""
            if len(sw.pending) <= PENDING_DEPTH:
                return
            if (
                len(sw.pending) < 7
                and pe_t < av_free_est[0]
                and sw.av is None
            ):
                return
            emit_pending_av(sw, stop=False)

        def close_segment(sw):
            """Evict/accumulate the open segment (or finish the sweep)."""
            final = sw.next_m == SM and bool(sw.pending)
            while sw.pending:
                emit_pending_av(sw, stop=len(sw.pending) == 1)
            av_free_est[0] = pe_t + CLOSE_LATENCY
            if sw.av is None:
                return
            av0, av1 = sw.av
            sw.av = None
            j, d = sw.j, sw.d
            if final and sw.pacc is None:
                # single full segment: normalize straight from psum (DVE)
                norm(j, d, (av0[:, 0:260], av1[:, 0:260]), sbuf_src=False)
                sw.done = True
                return
            if sw.pacc is None:
                sw.pacc = paccp.tile(
                    [128, 520], F32, tag=f"pacc{j}{d}", name=f"pacc{j}{d}"
                )
                nc.vector.tensor_copy(sw.pacc[:, 0:260], av0[:, 0:260])
                nc.vector.tensor_copy(sw.pacc[:, 260:520], av1[:, 0:260])
            else:
                nc.vector.tensor_add(sw.pacc[:, 0:260], sw.pacc[:, 0:260], av0[:, 0:260])
                nc.vector.tensor_add(sw.pacc[:, 260:520], sw.pacc[:, 260:520], av1[:, 0:260])
            if final:
                norm(j, d, (sw.pacc[:, 0:260], sw.pacc[:, 260:520]), sbuf_src=True)
                sw.done = True

        def norm(j, d, halves, sbuf_src=False):
            if j not in att_state:
                att_state[j] = [
                    attp.tile([128, JBLK], BF, tag=f"att{qc}", name=f"att{qc}")
                    for qc in range(4)
                ]
            att_j = att_state[j]
            # Pool can't read PSUM; only pacc (SBUF) sources may use it
            mul_eng = nc.gpsimd if sbuf_src else nc.vector
            for hh in range(2):
                src = halves[hh]
                h2 = 2 * d + hh
                for qc in range(4):
                    s0 = qc * 65
                    rcp = rcpp.tile([128, 1], F32, tag="rcp", name="rcp")
                    nc.vector.reciprocal(rcp[:], src[:, s0 + 64 : s0 + 65])
                    mul_eng.tensor_scalar_mul(
                        att_j[qc][:, h2 * 64 : (h2 + 1) * 64],
                        src[:, s0 : s0 + 64],
                        rcp[:],
                    )

        # ---- aux pieces (attT / wo) ----
        def attT_piece(j, pair, pool=None):
            pool = pool or prodp
            tag = "prod" if pool is prodp else "sc"
            att_j = att_state[j]
            attT_j = attT_state.setdefault(j, [None] * MD)
            tps = pool.tile([128, 1024], BF, tag=tag, name="atps")
            for half in range(2):
                kk = pair * 2 + half
                for qc in range(4):
                    nc.tensor.transpose(
                        tps[:, half * 512 + qc * 128 : half * 512 + (qc + 1) * 128],
                        att_j[qc][:, kk * 128 : (kk + 1) * 128],
                        identb_t[:],
                    )
            for half in range(2):
                kk = pair * 2 + half
                t = attTp.tile([128, JBLK], BF, tag=f"attT{kk}", name="attT")
                nc.vector.tensor_copy(t[:], tps[:, half * 512 : (half + 1) * 512])
                attT_j[kk] = t

        def wo_half(j, qc, pair):
            """Head-pairs 2*pair..2*pair+1 contribution to out[pair] rows."""
            attT_j = attT_state[j]
            wops = [
                prodp.tile([128, 512], F32, tag="prod", name=f"wops{nb}")
                for nb in range(2)
            ]
            for nb in range(2):
                for kk in (2 * pair, 2 * pair + 1):
                    nc.tensor.matmul(
                        wops[nb][:],
                        attT_j[kk][:, qc * 128 : (qc + 1) * 128],
                        wo_t[:, kk * D + nb * 512 : kk * D + nb * 512 + 512],
                        start=(kk == 2 * pair),
                        stop=(kk == 2 * pair + 1),
                    )
            ot = outp.tile([128, D], F32, tag="ot", name="ot")
            nc.vector.tensor_copy(ot[:, 0:512], wops[0][:])
            nc.vector.tensor_copy(ot[:, 512:1024], wops[1][:])
            mrow = j * JBLK + qc * 128
            nc.sync.dma_start(out=out[pair, mrow : mrow + 128, :], in_=ot[:])

        # ---- build-time scheduler ----
        # production queue: (kind, args, pe_cost)
        prod_q = deque()
        prod_q.append(("qp", (0, 0), EST_PROJ))
        prod_q.append(("k", (0,), EST_KPASS))
        prod_q.append(("qp", (0, 1), EST_PROJ))
        prod_q.append(("k", (1,), EST_KPASS + 1100.0))
        prod_q.append(("qp", (0, 2), EST_PROJ))
        prod_q.append(("qp", (0, 3), EST_PROJ))
        prod_q.append(("qt", (0,), 900.0))
        prod_q.append(("v", (0,), EST_VPASS))
        prod_q.append(("v", (1,), EST_VPASS))
        for m in range(2, SM):
            if m + 4 < SM:
                prod_q.append(("hdma", (m + 4,), 0.0))
            kcost = EST_KPASS + (1100.0 if m % TG == TG - 1 else 0.0)
            prod_q.append(("k", (m,), kcost))
            if 4 <= m <= 7:
                # j1 q-proj shares h chunks 4-7; must precede v(m) which
                # pops them
                prod_q.append(("qp", (1, m - 4), EST_PROJ))
            prod_q.append(("v", (m,), EST_VPASS))
            if m == 7:
                prod_q.append(("qt", (1,), 900.0))
        for j in (2, 3):
            prod_q.append(("qdma", (j, 0), 0.0))
            prod_q.append(("qdma", (j, 1), 0.0))
            for k in range(4):
                if k + 2 < 4:
                    prod_q.append(("qdma", (j, k + 2), 0.0))
                prod_q.append(("qp", (j, k), EST_PROJ))
            prod_q.append(("qt", (j,), 900.0))

        produced_k = set()
        produced_v = set()
        qt_ready = set()

        def run_prod(item):
            kind, args, _ = item
            if kind == "k":
                k_pass(*args)
                m = args[0]
                if m % TG == TG - 1:
                    for mm in range(m - TG + 1, m + 1):
                        produced_k.add(mm)
            elif kind == "v":
                v_pass(*args)
                produced_v.add(args[0])
            elif kind == "qp":
                q_pass(*args)
            elif kind == "qt":
                q_trans(*args)
                qt_ready.add(args[0])
            elif kind == "hdma":
                dma_h(*args)
            elif kind == "qdma":
                j, k = args
                q_dma(j, k)

        sweeps = [Sweep(j, d) for j in range(NJ) for d in range(MD)]
        aux_q = deque()  # (kind, args, pe_cost), dependency-ready aux work
        norm_count = {}  # j -> number of normed sweeps

        def on_sweep_done(sw):
            j = sw.j
            norm_count[j] = norm_count.get(j, 0) + 1
            dlist = [s.d for s in sweeps if s.j == j and s.done]
            for pair in range(2):
                if (
                    2 * pair in dlist
                    and 2 * pair + 1 in dlist
                    and (f"attT{pair}", j) not in emitted_aux
                ):
                    emitted_aux.add((f"attT{pair}", j))
                    # last attT of the kernel: sc ring is idle, borrow it
                    borrow_sc = j == NJ - 1 and pair == 1
                    aux_q.append(("attT", (j, pair, borrow_sc), 500.0))

        emitted_aux = set()

        def run_aux(item):
            kind, args, _ = item
            if kind == "attT":
                j, pair, borrow_sc = args
                attT_piece(j, pair, pool=scp if borrow_sc else prodp)
                for qc in range(4):
                    aux_q.append(("woh", (j, qc, pair), 950.0))
            elif kind == "woh":
                wo_half(*args)

        def sweep_ready(sw):
            return (
                not sw.done
                and sw.next_m < SM
                and sw.j in qt_ready
                and sw.next_m in produced_k
                and sw.next_m in produced_v
            )

        pe_t = 0.0
        act_t = 0.0
        active = None

        def emit_one_iter(sw):
            nonlocal pe_t, act_t, active
            if active is not None and active is not sw:
                close_segment(active)
                pe_t += 100.0
            active = sw
            emit_iter(sw)
            pe_t += EST_ITER_PE
            act_t = max(act_t + EST_ITER_ACT, pe_t + EST_SC + EST_ITER_ACT)

        def pick_sweep():
            if active is not None and sweep_ready(active):
                return active
            ready = [sw for sw in sweeps if sweep_ready(sw)]
            if not ready:
                return None

            # most available work first to minimize segment switches
            def avail(sw):
                hi = sw.next_m
                while hi < SM and hi in produced_k and hi in produced_v:
                    hi += 1
                return hi - sw.next_m

            return max(ready, key=avail)

        while True:
            remaining = [sw for sw in sweeps if not sw.done]
            if not remaining and not prod_q and not aux_q:
                break
            backlog = act_t - pe_t
            cand = pick_sweep() if backlog < BACKLOG_LO else None
            if cand is None and not prod_q and not aux_q:
                cand = pick_sweep()  # ACT-bound tail: keep sweeping
            if cand is not None:
                emit_one_iter(cand)
                if cand.next_m == SM:
                    close_segment(cand)
                    if cand is active:
                        active = None
                    on_sweep_done(cand)
                continue
            if prod_q:
                item = prod_q.popleft()
                run_prod(item)
                pe_t += item[2]
                continue
            if aux_q:
                item = aux_q.popleft()
                run_aux(item)
                pe_t += item[2]
                continue
            raise RuntimeError(
                f"scheduler deadlock: {[(sw.j, sw.d, sw.next_m) for sw in remaining]}"
            )
        assert all(sw.done for sw in sweeps)


def _wtile(w, dtype):
    """[K*128, N] -> [128, K*N] with tile[p, kk*N+c] = w[128*kk+p, c]."""
    kchunks = w.shape[0] // 128
    return np.ascontiguousarray(
        w.reshape(kchunks, 128, w.shape[1])
        .transpose(1, 0, 2)
        .reshape(128, kchunks * w.shape[1])
        .astype(dtype)
    )


def prep_core_inputs(positions, hidden_states, wq, wk, wv, wo):
    """Host-side sharding/pre-tiling. Returns list of 8 in_maps."""
    import ml_dtypes

    bf16 = ml_dtypes.bfloat16
    pos = np.asarray(positions).astype(np.float32)
    inv_freq = 1.0 / (ROPE_THETA ** (np.arange(0, DH, 2, dtype=np.float32) / DH))
    ang = pos[:, None] * inv_freq[None, :]  # [S, 32]
    cos_half = np.cos(ang).astype(np.float32)
    sin_half = np.sin(ang).astype(np.float32)
    cos_full = np.concatenate([cos_half, cos_half], axis=1)  # [S, 64]
    sin_signed = np.concatenate([-sin_half, sin_half], axis=1)  # [S, 64]
    cos_tiled = np.ascontiguousarray(
        cos_full.reshape(SM, 128, DH).transpose(1, 0, 2).reshape(128, SM * DH)
    )
    sin_tiled = np.ascontiguousarray(
        sin_signed.reshape(SM, 128, DH).transpose(1, 0, 2).reshape(128, SM * DH)
    )
    identb = np.eye(128, dtype=np.float32).astype(bf16)

    hs = np.asarray(hidden_states, dtype=np.float32)
    # hT[m, p, kk*128+cc] = hidden[b, 128m+cc, 128kk+p]
    hT_all = []
    for b in range(B):
        t = hs[b].reshape(SM, 128, KD, 128).transpose(0, 3, 2, 1)  # (m, p, kk, cc)
        hT_all.append(np.ascontiguousarray(t.reshape(SM, 128, D)).astype(bf16))

    in_maps = []
    for c in range(NCORES):
        b = c // 2
        g = c % 2
        sl = slice(g * MC, (g + 1) * MC)
        in_maps.append(
            {
                "hT": hT_all[b],
                "wq": _wtile(np.asarray(wq, np.float32)[:, sl], bf16),
                "wk": _wtile(np.asarray(wk, np.float32)[:, sl], bf16),
                "wv": _wtile(np.asarray(wv, np.float32)[:, sl], bf16),
                "wo": _wtile(np.asarray(wo, np.float32)[sl, :], bf16),
                "cosq": cos_tiled,
                "sinq": sin_tiled,
                "identb": identb,
            }
        )
    return in_maps


_NC_CACHE = {}


def get_nc():
    if "nc" not in _NC_CACHE:
        _NC_CACHE["nc"] = build_nc()
    return _NC_CACHE["nc"]


def kernel(positions, hidden_states, wq, wk, wv, wo):
    in_dtype = np.asarray(hidden_states).dtype
    in_maps = prep_core_inputs(positions, hidden_states, wq, wk, wv, wo)
    nc = get_nc()
    res = bass_utils.run_bass_kernel_spmd(nc, in_maps, core_ids=list(range(NCORES)))
    outs = np.empty((B, S, D), dtype=np.float32)
    for b in range(B):
        o0 = res.results[2 * b]["out"]
        o1 = res.results[2 * b + 1]["out"]
        outs[b] = (o0[0] + o0[1]) + (o1[0] + o1[1])
    return outs.astype(in_dtype, copy=False)


# revision 92
# speedup vs baseline: 1.1883x; 1.0528x over previous
"""Trainium2 Bass kernel for Chronos2Attention (B=4, S=2048, D=1024, H=16, Dh=64).

Sharding: 8 cores = 4 batches x 2 head-groups. Core c handles batch c//2 and
heads 8*(c%2) .. 8*(c%2)+7 (wq/wk/wv column-sharded, wo row-sharded); host sums
the two partial [S, D] outputs per batch at gather time.

v4 design (vs v3): fully fused production/attention pipeline.
  - All PE-side tensors bf16 (h, weights, roped q/k, kt, v, attn) -> every
    matmul/transpose runs at 1 cyc/row; halved DMA.
  - K/V/Q production streams through a 2-buf [128,512]f32 PSUM "prod" pool
    just-in-time; attention sweeps consume chunks as they appear. j0's Q
    projections borrow the (still idle) av banks so K can start in parallel;
    head k-ropes run on Pool to keep the DVE queue short.
  - Sweeps (j = 512-query block, d = head pair) are SEGMENTED: AV accumulates
    in a single PSUM bank-pair per segment (2 heads x 4qc x 65 cols with the
    ones/denominator column), partial segments accumulated into SBUF f32
    "pacc" tiles so 2 av banks suffice while many sweeps are in flight
    against a limited set of produced chunks.
  - AV matmuls are emitted LAZILY (deque per sweep, depth-capped by the
    ex-tile ring) so exps can run ahead across av-bank handoffs; the stop
    flag lands on the true last matmul of each segment.
  - Emission is availability-driven: a tiny cost model interleaves production
    passes, sweep iterations (sc->exp->AV), and aux work (norm/attT/wo/q-proj)
    to keep ACT (exp, the ~266us floor) saturated and PE dense. Engines:
    ACT = exp only (anything else deadlock-prone via ring WARs); DVE = ropes,
    psum evictions, pacc, norms-from-psum; Pool = head ropes + norms-from-pacc.
  - Wo is computed in head-pair halves into out[2, S, D] (host sums 4 partial
    outputs per batch) so each j block's Wo work drains early; the final
    block's last attT borrows the then-idle sc ring.
  - PSUM budget: sc ring 2x[128,1024] (4 banks) + av 2x[128,512] (2) +
    prod 2x[128,512] (2) = 8 banks.
"""

from collections import deque

import numpy as np

import concourse.bacc as bacc
import concourse.mybir as mybir
import concourse.tile as tile
from concourse import bass_utils

# Problem shapes (hardcoded per spec)
B = 4
S = 2048
D = 1024
H = 16
DH = 64
ROPE_THETA = 10000.0
NCORES = 8
HC = H // 2  # heads per core
MC = HC * DH  # 512, per-core projection width

SM = S // 128  # 16 seq chunks
KD = D // 128  # 8 contraction chunks for projections
MD = HC // 2  # 4 head-pair sweeps per j block
JBLK = 512
NJ = S // JBLK  # 4
TG = 2  # k-transpose group size

F32 = mybir.dt.float32
BF = mybir.dt.bfloat16

# emission cost estimates (ns) for the build-time scheduler
EST_PROJ = 1800.0
EST_KPASS = 2800.0  # proj + rope serialization through the prod ring
EST_VPASS = 2200.0
EST_ITER_PE = 700.0
EST_ITER_ACT = 1045.0
EST_SC = 450.0
BACKLOG_LO = 6000.0  # emit sweep iters while ACT backlog below this


def build_nc():
    nc = bacc.Bacc("TRN2", target_bir_lowering=False, debug=False, num_devices=1)

    hT = nc.dram_tensor("hT", [SM, 128, D], BF, kind="ExternalInput").ap()
    wq = nc.dram_tensor("wq", [128, KD * MC], BF, kind="ExternalInput").ap()
    wk = nc.dram_tensor("wk", [128, KD * MC], BF, kind="ExternalInput").ap()
    wv = nc.dram_tensor("wv", [128, KD * MC], BF, kind="ExternalInput").ap()
    wo = nc.dram_tensor("wo", [128, MD * D], BF, kind="ExternalInput").ap()
    cosq = nc.dram_tensor("cosq", [128, SM * DH], F32, kind="ExternalInput").ap()
    sinq = nc.dram_tensor("sinq", [128, SM * DH], F32, kind="ExternalInput").ap()
    identb = nc.dram_tensor("identb", [128, 128], BF, kind="ExternalInput").ap()
    # out[ph]: contribution of head-pairs 2ph..2ph+1 (summed host-side)
    out = nc.dram_tensor("out", [2, S, D], F32, kind="ExternalOutput").ap()

    with tile.TileContext(nc) as tc:
        _build_body(nc, tc, hT, wq, wk, wv, wo, cosq, sinq, identb, out)
    nc.compile()
    return nc


def _build_body(nc, tc, hT, wq, wk, wv, wo, cosq, sinq, identb, out):
    from contextlib import ExitStack

    Exp = mybir.ActivationFunctionType.Exp

    with ExitStack() as ctx:
        # ---- persistent SBUF ----
        persist = ctx.enter_context(tc.tile_pool(name="persist", bufs=1))
        kt = [persist.tile([128, S], BF, tag=f"kt{d}", name=f"kt{d}") for d in range(MD)]
        # v1[p, m*520 + h*65 + e]: e<64 -> V dims, e==64 -> ones (softmax denom)
        v1 = persist.tile([128, SM * (HC * 65)], BF, tag="v1", name="v1")
        wq_t = persist.tile([128, KD * MC], BF, tag="w_q", name="w_q")
        wk_t = persist.tile([128, KD * MC], BF, tag="w_k", name="w_k")
        wv_t = persist.tile([128, KD * MC], BF, tag="w_v", name="w_v")
        wo_t = persist.tile([128, MD * D], BF, tag="wo", name="wo_t")
        cos_t = persist.tile([128, SM * DH], F32, tag="cos", name="cos_t")
        sin_t = persist.tile([128, SM * DH], F32, tag="sin", name="sin_t")
        identb_t = persist.tile([128, 128], BF, tag="identb", name="identb_t")

        # ---- working SBUF pools ----
        hpool = ctx.enter_context(tc.tile_pool(name="hprod", bufs=8))
        hqpool = ctx.enter_context(tc.tile_pool(name="hq", bufs=4))
        rkpool = ctx.enter_context(tc.tile_pool(name="rk", bufs=4))
        kfpool = ctx.enter_context(tc.tile_pool(name="kf", bufs=2))
        rqpool = ctx.enter_context(tc.tile_pool(name="rq", bufs=4))
        qpool = ctx.enter_context(tc.tile_pool(name="qtj", bufs=3))
        tmpp = ctx.enter_context(tc.tile_pool(name="ropetmp", bufs=3))
        expp = ctx.enter_context(tc.tile_pool(name="expp", bufs=9))
        paccp = ctx.enter_context(tc.tile_pool(name="pacc", bufs=1))
        rcpp = ctx.enter_context(tc.tile_pool(name="rcpp", bufs=8))
        attp = ctx.enter_context(tc.tile_pool(name="attp", bufs=3))
        attTp = ctx.enter_context(tc.tile_pool(name="attTp", bufs=3))
        outp = ctx.enter_context(tc.tile_pool(name="outp", bufs=2))

        # ---- PSUM: 4 + 2 + 2 = 8 banks ----
        scp = ctx.enter_context(tc.tile_pool(name="scp", bufs=2, space="PSUM"))
        avp = ctx.enter_context(tc.tile_pool(name="avp", bufs=2, space="PSUM"))
        prodp = ctx.enter_context(tc.tile_pool(name="prodp", bufs=2, space="PSUM"))

        # ---- init: DMAs ordered for earliest first sweep ----
        h_tiles = {}

        def dma_h(m):
            t = hpool.tile([128, D], BF, tag="h", name="h_m")
            nc.sync.dma_start(out=t[:], in_=hT[m])
            h_tiles[m] = t

        half = 4 * MC
        csh = SM * DH // 2
        nc.sync.dma_start(out=wq_t[:, 0:half], in_=wq[:, 0:half])
        nc.sync.dma_start(out=cos_t[:, 0:csh], in_=cosq[:, 0:csh])
        nc.sync.dma_start(out=sin_t[:, 0:csh], in_=sinq[:, 0:csh])
        dma_h(0)
        dma_h(1)
        nc.sync.dma_start(out=wq_t[:, half:], in_=wq[:, half:])
        nc.sync.dma_start(out=wk_t[:, 0:half], in_=wk[:, 0:half])
        nc.sync.dma_start(out=wk_t[:, half:], in_=wk[:, half:])
        nc.sync.dma_start(out=wv_t[:, 0:half], in_=wv[:, 0:half])
        nc.sync.dma_start(out=identb_t[:], in_=identb)
        dma_h(2)
        dma_h(3)
        nc.sync.dma_start(out=cos_t[:, csh:], in_=cosq[:, csh:])
        nc.sync.dma_start(out=sin_t[:, csh:], in_=sinq[:, csh:])
        nc.sync.dma_start(out=wv_t[:, half:], in_=wv[:, half:])
        dma_h(4)
        dma_h(5)
        dma_h(6)
        dma_h(7)
        nc.sync.dma_start(out=wo_t[:], in_=wo)

        # PE warmup (p-state ramp) on dep-free memset tiles
        wu = persist.tile([128, 512], F32, tag="wu", name="wu")
        nc.vector.memset(wu[:], 0.0)
        wups = prodp.tile([128, 512], F32, tag="prod", name="wups")
        for i in range(2):
            nc.tensor.matmul(
                wups[:], wu[:, 0:128], wu[:], start=(i == 0), stop=(i == 1)
            )

        # ones columns of v1 (Pool memset + DVE broadcast); warm the exp table
        oneb = persist.tile([128, 1], BF, tag="oneb", name="oneb")
        nc.gpsimd.memset(oneb[:], 1.0)
        nc.vector.tensor_copy(
            v1[:].rearrange("p (m h e) -> p m h e", m=SM, h=HC)[:, :, :, 64:65],
            oneb[:, None, None, 0:1].broadcast_to([128, SM, HC, 1]),
        )
        warm = persist.tile([1, 16], F32, tag="warm", name="warm")
        nc.vector.memset(warm[:], 0.0)
        nc.scalar.activation(warm[:], warm[:], Exp)

        # ---- shared helpers ----
        def rope(src, r, m, eng):
            """Natural-layout RoPE: src [128 seq, HC*DH] -> r bf16.
            eng=nc.vector reads PSUM directly (low latency, K path);
            eng=nc.gpsimd needs an SBUF source (Q path, off critical path)."""
            cos_m = cos_t[:, None, m * DH : (m + 1) * DH]
            sin_m = sin_t[:, m * DH : (m + 1) * DH]
            tc_ = tmpp.tile([128, MC], F32, tag="tc", name="tc_")
            ts_ = tmpp.tile([128, MC], F32, tag="ts", name="ts_")
            p3 = src.rearrange("p (h e) -> p h e", h=HC)
            t3 = ts_[:].rearrange("p (h e) -> p h e", h=HC)
            eng.tensor_mul(
                tc_[:].rearrange("p (h e) -> p h e", h=HC),
                p3,
                cos_m.broadcast_to([128, HC, DH]),
            )
            eng.tensor_mul(
                t3[:, :, 0:32],
                p3[:, :, 32:64],
                sin_m[:, None, 0:32].broadcast_to([128, HC, 32]),
            )
            eng.tensor_mul(
                t3[:, :, 32:64],
                p3[:, :, 0:32],
                sin_m[:, None, 32:64].broadcast_to([128, HC, 32]),
            )
            eng.tensor_add(r[:], tc_[:], ts_[:])

        def proj(h_m, w_t, pool=None, tag="prod"):
            ps = (pool or prodp).tile([128, MC], F32, tag=tag, name="ps")
            for kk in range(KD):
                nc.tensor.matmul(
                    ps[:],
                    h_m[:, kk * 128 : (kk + 1) * 128],
                    w_t[:, kk * MC : (kk + 1) * MC],
                    start=(kk == 0),
                    stop=(kk == KD - 1),
                )
            return ps

        # ---- production pieces ----
        rot_k = [None] * TG
        qrot = {j: [None] * 4 for j in range(NJ)}
        qtj = {}

        def k_pass(m):
            ps = proj(h_tiles[m], wk_t)
            r = rkpool.tile([128, MC], BF, tag="rk", name="rk")
            if m < 2:
                # head: keep the DVE queue free for q-ropes; rope on Pool
                kf = kfpool.tile([128, MC], F32, tag="kf", name="kf")
                nc.vector.tensor_copy(kf[:], ps[:])
                rope(kf[:], r, m, nc.gpsimd)
            else:
                rope(ps[:], r, m, nc.vector)
            rot_k[m % TG] = r
            if m % TG == TG - 1:
                m0 = m - (TG - 1)
                W = TG * 128
                for dpair in range(2):
                    tps = prodp.tile([128, 2 * W], BF, tag="prod", name="ktps")
                    for half in range(2):
                        d = dpair * 2 + half
                        for mm in range(TG):
                            nc.tensor.transpose(
                                tps[:, half * W + mm * 128 : half * W + (mm + 1) * 128],
                                rot_k[mm][:, d * 128 : (d + 1) * 128],
                                identb_t[:],
                            )
                    for half in range(2):
                        d = dpair * 2 + half
                        nc.vector.tensor_copy(
                            kt[d][:, m0 * 128 : m0 * 128 + W],
                            tps[:, half * W : (half + 1) * W],
                        )

        def v_pass(m):
            ps = proj(h_tiles[m], wv_t)
            dst = v1[:, m * (HC * 65) : (m + 1) * (HC * 65)].rearrange(
                "p (h e) -> p h e", h=HC
            )[:, :, 0:64]
            nc.vector.tensor_copy(dst, ps[:].rearrange("p (h e) -> p h e", h=HC))
            h_tiles.pop(m)

        def q_pass(j, k):
            m = j * 4 + k
            if j == 0:
                h_m = h_tiles[m]
            else:
                h_m = h_tiles.pop(("q", m))
            # j0's projections borrow the av banks (idle until the first AV)
            # so K production streams through prodp in parallel
            ps = proj(h_m, wq_t, pool=avp if j == 0 else None,
                      tag="av" if j == 0 else "prod")
            r = rqpool.tile([128, MC], BF, tag="rq", name="rq")
            rope(ps[:], r, m, nc.vector)
            qrot[j][k] = r

        def q_dma(j, k):
            m = j * 4 + k
            t = hqpool.tile([128, D], BF, tag="hq", name="hq")
            nc.sync.dma_start(out=t[:], in_=hT[m])
            h_tiles[("q", m)] = t

        def q_trans(j):
            tiles = [
                qpool.tile([128, JBLK], BF, tag=f"qt{d}", name=f"qt{d}")
                for d in range(MD)
            ]
            for dpair in range(2):
                tps = prodp.tile([128, 1024], BF, tag="prod", name="qtps")
                for half in range(2):
                    d = dpair * 2 + half
                    for mm in range(4):
                        nc.tensor.transpose(
                            tps[:, half * 512 + mm * 128 : half * 512 + (mm + 1) * 128],
                            qrot[j][mm][:, d * 128 : (d + 1) * 128],
                            identb_t[:],
                        )
                for half in range(2):
                    d = dpair * 2 + half
                    nc.vector.tensor_copy(
                        tiles[d][:], tps[:, half * 512 : (half + 1) * 512]
                    )
            qrot[j] = [None] * 4
            qtj[j] = tiles

        # ---- sweep machinery ----
        # state per sweep (j,d): next_m, av pair or None, pacc tile, pending AV
        att_state = {}  # j -> [att tiles per qc]
        attT_state = {}  # j -> [attT tiles per kk]

        PENDING_DEPTH = 2  # min AV emission lag behind exp (iters)
        CLOSE_LATENCY = 2600.0  # est ns from close emission to av-bank free

        class Sweep:
            def __init__(self, j, d):
                self.j = j
                self.d = d
                self.next_m = 0
                self.av = None  # (av0, av1) psum pair while segment open
                self.seg_start = True  # next pending AV opens a segment
                self.pacc = None
                self.pending = deque()  # (m, ex) awaiting AV emission
                self.done = False

        def emit_pending_av(sw, stop):
            m, ex = sw.pending.popleft()
            if sw.av is None:
                sw.av = (
                    avp.tile([128, 512], F32, tag="av", name="av0"),
                    avp.tile([128, 512], F32, tag="av", name="av1"),
                )
                sw.seg_start = True
            for hh in range(2):
                h2 = 2 * sw.d + hh
                vs = m * (HC * 65) + h2 * 65
                for qc in range(4):
                    nc.tensor.matmul(
                        sw.av[hh][:, qc * 65 : (qc + 1) * 65],
                        ex[:, hh * JBLK + qc * 128 : hh * JBLK + (qc + 1) * 128],
                        v1[:, vs : vs + 65],
                        start=(sw.seg_start and qc == 0),
                        stop=(stop and qc == 3),
                    )
            sw.seg_start = False

        def emit_iter(sw):
            """One m iteration: sc matmuls + exp; AV of previous iter."""
            m = sw.next_m
            sw.next_m += 1
            d, j = sw.d, sw.j
            sc = scp.tile([128, 2 * JBLK], F32, tag="sc", name="sc")
            nc.tensor.matmul(
                sc[:, 0:JBLK],
                kt[d][0:64, m * 128 : (m + 1) * 128],
                qtj[j][d][0:64, :],
                start=True,
                stop=True,
            )
            nc.tensor.matmul(
                sc[:, JBLK : 2 * JBLK],
                kt[d][64:128, m * 128 : (m + 1) * 128],
                qtj[j][d][64:128, :],
                start=True,
                stop=True,
            )
            drain_pending(sw)
            ex = expp.tile([128, 2 * JBLK], BF, tag="ex", name="ex")
            nc.scalar.activation(ex[:], sc[:], Exp)
            sw.pending.append((m, ex))

        av_free_est = [0.0]

        def drain_pending(sw):
            """Emit one deferred AV unless a new segment would have to wait
            for the av pair to drain (let exps run ahead instead). Hard cap:
            an exp's ex-tile allocation WARs against the AV 10 allocations
            back (expp ring) — if AVs lag more than ring-2, the sem graph
            deadlocks (exp waits AV, AV behind a blocked sc, sc waits exp)."""
            if len(sw.pending) <= PENDING_DEPTH:
                return
            if (
                len(sw.pending) < 7
                and pe_t < av_free_est[0]
                and sw.av is None
            ):
                return
            emit_pending_av(sw, stop=False)

        def close_segment(sw):
            """Evict/accumulate the open segment (or finish the sweep)."""
            final = sw.next_m == SM and bool(sw.pending)
            while sw.pending:
                emit_pending_av(sw, stop=len(sw.pending) == 1)
            av_free_est[0] = pe_t + CLOSE_LATENCY
            if sw.av is None:
                return
            av0, av1 = sw.av
            sw.av = None
            j, d = sw.j, sw.d
            if final and sw.pacc is None:
                # single full segment: normalize straight from psum (DVE)
                post = all(s.done for s in sweeps if s is not sw)
                norm(j, d, (av0[:, 0:260], av1[:, 0:260]), sbuf_src=False,
                     post_exp=post)
                sw.done = True
                return
            if sw.pacc is None:
                sw.pacc = paccp.tile(
                    [128, 520], F32, tag=f"pacc{j}{d}", name=f"pacc{j}{d}"
                )
                nc.vector.tensor_copy(sw.pacc[:, 0:260], av0[:, 0:260])
                nc.vector.tensor_copy(sw.pacc[:, 260:520], av1[:, 0:260])
            else:
                nc.vector.tensor_add(sw.pacc[:, 0:260], sw.pacc[:, 0:260], av0[:, 0:260])
                nc.vector.tensor_add(sw.pacc[:, 260:520], sw.pacc[:, 260:520], av1[:, 0:260])
            if final:
                post = all(s.done for s in sweeps if s is not sw)
                norm(j, d, (sw.pacc[:, 0:260], sw.pacc[:, 260:520]), sbuf_src=True,
                     post_exp=post)
                sw.done = True

        def norm(j, d, halves, sbuf_src=False, post_exp=False):
            if j not in att_state:
                att_state[j] = [
                    attp.tile([128, JBLK], BF, tag=f"att{qc}", name=f"att{qc}")
                    for qc in range(4)
                ]
            att_j = att_state[j]
            # Pool can't read PSUM; only pacc (SBUF) sources may use it
            mul_eng = nc.gpsimd if sbuf_src else nc.vector
            Copy = mybir.ActivationFunctionType.Copy
            for hh in range(2):
                src = halves[hh]
                h2 = 2 * d + hh
                for qc in range(4):
                    s0 = qc * 65
                    rcp = rcpp.tile([128, 1], F32, tag="rcp", name="rcp")
                    nc.vector.reciprocal(rcp[:], src[:, s0 + 64 : s0 + 65])
                    dst = att_j[qc][:, h2 * 64 : (h2 + 1) * 64]
                    mul_eng.tensor_scalar_mul(dst, src[:, s0 : s0 + 64], rcp[:])

        # ---- aux pieces (attT / wo) ----
        def attT_piece(j, pair, pool=None):
            pool = pool or prodp
            tag = "prod" if pool is prodp else "sc"
            att_j = att_state[j]
            attT_j = attT_state.setdefault(j, [None] * MD)
            tps = pool.tile([128, 1024], BF, tag=tag, name="atps")
            for half in range(2):
                kk = pair * 2 + half
                for qc in range(4):
                    nc.tensor.transpose(
                        tps[:, half * 512 + qc * 128 : half * 512 + (qc + 1) * 128],
                        att_j[qc][:, kk * 128 : (kk + 1) * 128],
                        identb_t[:],
                    )
            for half in range(2):
                kk = pair * 2 + half
                t = attTp.tile([128, JBLK], BF, tag=f"attT{kk}", name="attT")
                nc.vector.tensor_copy(t[:], tps[:, half * 512 : (half + 1) * 512])
                attT_j[kk] = t

        def wo_half(j, qc, pair):
            """Head-pairs 2*pair..2*pair+1 contribution to out[pair] rows."""
            attT_j = attT_state[j]
            wops = [
                prodp.tile([128, 512], F32, tag="prod", name=f"wops{nb}")
                for nb in range(2)
            ]
            for nb in range(2):
                for kk in (2 * pair, 2 * pair + 1):
                    nc.tensor.matmul(
                        wops[nb][:],
                        attT_j[kk][:, qc * 128 : (qc + 1) * 128],
                        wo_t[:, kk * D + nb * 512 : kk * D + nb * 512 + 512],
                        start=(kk == 2 * pair),
                        stop=(kk == 2 * pair + 1),
                    )
            ot = outp.tile([128, D], F32, tag="ot", name="ot")
            mrow = j * JBLK + qc * 128
            if all(s.done for s in sweeps):
                # tail: chain each half's DMA behind its eviction
                for nb in range(2):
                    nc.vector.tensor_copy(ot[:, nb * 512 : (nb + 1) * 512], wops[nb][:])
                    nc.sync.dma_start(
                        out=out[pair, mrow : mrow + 128, nb * 512 : (nb + 1) * 512],
                        in_=ot[:, nb * 512 : (nb + 1) * 512],
                    )
            else:
                nc.vector.tensor_copy(ot[:, 0:512], wops[0][:])
                nc.vector.tensor_copy(ot[:, 512:1024], wops[1][:])
                nc.sync.dma_start(out=out[pair, mrow : mrow + 128, :], in_=ot[:])

        # ---- build-time scheduler ----
        # production queue: (kind, args, pe_cost)
        prod_q = deque()
        prod_q.append(("qp", (0, 0), EST_PROJ))
        prod_q.append(("k", (0,), EST_KPASS))
        prod_q.append(("qp", (0, 1), EST_PROJ))
        prod_q.append(("k", (1,), EST_KPASS + 1100.0))
        prod_q.append(("qp", (0, 2), EST_PROJ))
        prod_q.append(("qp", (0, 3), EST_PROJ))
        prod_q.append(("qt", (0,), 900.0))
        prod_q.append(("k", (2,), EST_KPASS))
        prod_q.append(("v", (0,), EST_VPASS))
        prod_q.append(("k", (3,), EST_KPASS + 1100.0))
        prod_q.append(("v", (1,), EST_VPASS))
        prod_q.append(("k", (4,), EST_KPASS))
        prod_q.append(("k", (5,), EST_KPASS + 1100.0))
        prod_q.append(("k", (6,), EST_KPASS))
        prod_q.append(("k", (7,), EST_KPASS + 1100.0))
        for m in range(2, SM):
            if m + 6 < SM:
                prod_q.append(("hdma", (m + 6,), 0.0))
            if m >= 8:
                # k staggered ahead so kt groups land early
                kcost = EST_KPASS + (1100.0 if m % TG == TG - 1 else 0.0)
                prod_q.append(("k", (m,), kcost))
            if 4 <= m <= 7:
                prod_q.append(("qdma", (1, m - 4), 0.0))
            if 6 <= m <= 9:
                prod_q.append(("qp", (1, m - 6), EST_PROJ))
            if 8 <= m <= 11:
                prod_q.append(("qdma", (2, m - 8), 0.0))
            if 10 <= m <= 13:
                prod_q.append(("qp", (2, m - 10), EST_PROJ))
            prod_q.append(("v", (m,), EST_VPASS))
            if m == 9:
                prod_q.append(("qt", (1,), 900.0))
            if m == 13:
                prod_q.append(("qt", (2,), 900.0))
        for j in (3,):
            prod_q.append(("qdma", (j, 0), 0.0))
            prod_q.append(("qdma", (j, 1), 0.0))
            for k in range(4):
                if k + 2 < 4:
                    prod_q.append(("qdma", (j, k + 2), 0.0))
                prod_q.append(("qp", (j, k), EST_PROJ))
            prod_q.append(("qt", (j,), 900.0))

        produced_k = set()
        produced_v = set()
        qt_ready = set()

        def run_prod(item):
            kind, args, _ = item
            if kind == "k":
                k_pass(*args)
                m = args[0]
                if m % TG == TG - 1:
                    for mm in range(m - TG + 1, m + 1):
                        produced_k.add(mm)
            elif kind == "v":
                v_pass(*args)
                produced_v.add(args[0])
            elif kind == "qp":
                q_pass(*args)
            elif kind == "qt":
                q_trans(*args)
                qt_ready.add(args[0])
            elif kind == "hdma":
                dma_h(*args)
            elif kind == "qdma":
                j, k = args
                q_dma(j, k)

        sweeps = [Sweep(j, d) for j in range(NJ) for d in range(MD)]
        aux_q = deque()  # (kind, args, pe_cost), dependency-ready aux work
        norm_count = {}  # j -> number of normed sweeps

        def on_sweep_done(sw):
            j = sw.j
            norm_count[j] = norm_count.get(j, 0) + 1
            dlist = [s.d for s in sweeps if s.j == j and s.done]
            for pair in range(2):
                if (
                    2 * pair in dlist
                    and 2 * pair + 1 in dlist
                    and (f"attT{pair}", j) not in emitted_aux
                ):
                    emitted_aux.add((f"attT{pair}", j))
                    # last attT of the kernel: sc ring is idle, borrow it
                    borrow_sc = j == NJ - 1 and pair == 1
                    aux_q.append(("attT", (j, pair, borrow_sc), 500.0))

        emitted_aux = set()

        def run_aux(item):
            kind, args, _ = item
            if kind == "attT":
                j, pair, borrow_sc = args
                attT_piece(j, pair, pool=scp if borrow_sc else prodp)
                for qc in range(4):
                    aux_q.append(("woh", (j, qc, pair), 950.0))
            elif kind == "woh":
                wo_half(*args)

        def sweep_ready(sw):
            return (
                not sw.done
                and sw.next_m < SM
                and sw.j in qt_ready
                and sw.next_m in produced_k
                and sw.next_m in produced_v
            )

        pe_t = 0.0
        act_t = 0.0
        active = None

        def emit_one_iter(sw):
            nonlocal pe_t, act_t, active
            if active is not None and active is not sw:
                close_segment(active)
                pe_t += 100.0
            active = sw
            emit_iter(sw)
            pe_t += EST_ITER_PE
            act_t = max(act_t + EST_ITER_ACT, pe_t + EST_SC + EST_ITER_ACT)

        def pick_sweep():
            if active is not None and sweep_ready(active):
                return active
            ready = [sw for sw in sweeps if sweep_ready(sw)]
            if not ready:
                return None

            def avail(sw):
                hi = sw.next_m
                while hi < SM and hi in produced_k and hi in produced_v:
                    hi += 1
                return hi - sw.next_m

            # finish low-j blocks first (staggers attT/wo aux); among same j
            # prefer the sweep with most available chunks (fewer switches)
            return min(ready, key=lambda sw: (sw.j, -avail(sw)))

        while True:
            remaining = [sw for sw in sweeps if not sw.done]
            if not remaining and not prod_q and not aux_q:
                break
            backlog = act_t - pe_t
            cand = pick_sweep() if backlog < BACKLOG_LO else None
            if cand is None and not prod_q and not aux_q:
                cand = pick_sweep()  # ACT-bound tail: keep sweeping
            if cand is not None:
                emit_one_iter(cand)
                if cand.next_m == SM:
                    close_segment(cand)
                    if cand is active:
                        active = None
                    on_sweep_done(cand)
                continue
            if prod_q:
                item = prod_q.popleft()
                run_prod(item)
                pe_t += item[2]
                continue
            if aux_q:
                item = aux_q.popleft()
                run_aux(item)
                pe_t += item[2]
                continue
            raise RuntimeError(
                f"scheduler deadlock: {[(sw.j, sw.d, sw.next_m) for sw in remaining]}"
            )
        assert all(sw.done for sw in sweeps)


def _wtile(w, dtype):
    """[K*128, N] -> [128, K*N] with tile[p, kk*N+c] = w[128*kk+p, c]."""
    kchunks = w.shape[0] // 128
    return np.ascontiguousarray(
        w.reshape(kchunks, 128, w.shape[1])
        .transpose(1, 0, 2)
        .reshape(128, kchunks * w.shape[1])
        .astype(dtype)
    )


def prep_core_inputs(positions, hidden_states, wq, wk, wv, wo):
    """Host-side sharding/pre-tiling. Returns list of 8 in_maps."""
    import ml_dtypes

    bf16 = ml_dtypes.bfloat16
    pos = np.asarray(positions).astype(np.float32)
    inv_freq = 1.0 / (ROPE_THETA ** (np.arange(0, DH, 2, dtype=np.float32) / DH))
    ang = pos[:, None] * inv_freq[None, :]  # [S, 32]
    cos_half = np.cos(ang).astype(np.float32)
    sin_half = np.sin(ang).astype(np.float32)
    cos_full = np.concatenate([cos_half, cos_half], axis=1)  # [S, 64]
    sin_signed = np.concatenate([-sin_half, sin_half], axis=1)  # [S, 64]
    cos_tiled = np.ascontiguousarray(
        cos_full.reshape(SM, 128, DH).transpose(1, 0, 2).reshape(128, SM * DH)
    )
    sin_tiled = np.ascontiguousarray(
        sin_signed.reshape(SM, 128, DH).transpose(1, 0, 2).reshape(128, SM * DH)
    )
    identb = np.eye(128, dtype=np.float32).astype(bf16)

    hs = np.asarray(hidden_states, dtype=np.float32)
    # hT[m, p, kk*128+cc] = hidden[b, 128m+cc, 128kk+p]
    hT_all = []
    for b in range(B):
        t = hs[b].reshape(SM, 128, KD, 128).transpose(0, 3, 2, 1)  # (m, p, kk, cc)
        hT_all.append(np.ascontiguousarray(t.reshape(SM, 128, D)).astype(bf16))

    in_maps = []
    for c in range(NCORES):
        b = c // 2
        g = c % 2
        sl = slice(g * MC, (g + 1) * MC)
        in_maps.append(
            {
                "hT": hT_all[b],
                "wq": _wtile(np.asarray(wq, np.float32)[:, sl], bf16),
                "wk": _wtile(np.asarray(wk, np.float32)[:, sl], bf16),
                "wv": _wtile(np.asarray(wv, np.float32)[:, sl], bf16),
                "wo": _wtile(np.asarray(wo, np.float32)[sl, :], bf16),
                "cosq": cos_tiled,
                "sinq": sin_tiled,
                "identb": identb,
            }
        )
    return in_maps


_NC_CACHE = {}


def get_nc():
    if "nc" not in _NC_CACHE:
        _NC_CACHE["nc"] = build_nc()
    return _NC_CACHE["nc"]


def kernel(positions, hidden_states, wq, wk, wv, wo):
    in_dtype = np.asarray(hidden_states).dtype
    in_maps = prep_core_inputs(positions, hidden_states, wq, wk, wv, wo)
    nc = get_nc()
    res = bass_utils.run_bass_kernel_spmd(nc, in_maps, core_ids=list(range(NCORES)))
    outs = np.empty((B, S, D), dtype=np.float32)
    for b in range(B):
        o0 = res.results[2 * b]["out"]
        o1 = res.results[2 * b + 1]["out"]
        outs[b] = (o0[0] + o0[1]) + (o1[0] + o1[1])
    return outs.astype(in_dtype, copy=False)


# revision 97
# speedup vs baseline: 1.1905x; 1.0018x over previous
"""Trainium2 Bass kernel for Chronos2Attention (B=4, S=2048, D=1024, H=16, Dh=64).

Sharding: 8 cores = 4 batches x 2 head-groups. Core c handles batch c//2 and
heads 8*(c%2) .. 8*(c%2)+7 (wq/wk/wv column-sharded, wo row-sharded); host sums
the two partial [S, D] outputs per batch at gather time.

v4 design (vs v3): fully fused production/attention pipeline.
  - All PE-side tensors bf16 (h, weights, roped q/k, kt, v, attn) -> every
    matmul/transpose runs at 1 cyc/row; halved DMA.
  - K/V/Q production streams through a 2-buf [128,512]f32 PSUM "prod" pool
    just-in-time; attention sweeps consume chunks as they appear. j0's Q
    projections borrow the (still idle) av banks so K can start in parallel;
    head k-ropes run on Pool to keep the DVE queue short.
  - Sweeps (j = 512-query block, d = head pair) are SEGMENTED: AV accumulates
    in a single PSUM bank-pair per segment (2 heads x 4qc x 65 cols with the
    ones/denominator column), partial segments accumulated into SBUF f32
    "pacc" tiles so 2 av banks suffice while many sweeps are in flight
    against a limited set of produced chunks.
  - AV matmuls are emitted LAZILY (deque per sweep, depth-capped by the
    ex-tile ring) so exps can run ahead across av-bank handoffs; the stop
    flag lands on the true last matmul of each segment.
  - Emission is availability-driven: a tiny cost model interleaves production
    passes, sweep iterations (sc->exp->AV), and aux work (norm/attT/wo/q-proj)
    to keep ACT (exp, the ~266us floor) saturated and PE dense. Engines:
    ACT = exp only (anything else deadlock-prone via ring WARs); DVE = ropes,
    psum evictions, pacc, norms-from-psum; Pool = head ropes + norms-from-pacc.
  - Wo is computed in head-pair halves into out[2, S, D] (host sums 4 partial
    outputs per batch) so each j block's Wo work drains early; the final
    block's last attT borrows the then-idle sc ring.
  - PSUM budget: sc ring 2x[128,1024] (4 banks) + av 2x[128,512] (2) +
    prod 2x[128,512] (2) = 8 banks.
"""

from collections import deque

import numpy as np

import concourse.bacc as bacc
import concourse.mybir as mybir
import concourse.tile as tile
from concourse import bass_utils

# Problem shapes (hardcoded per spec)
B = 4
S = 2048
D = 1024
H = 16
DH = 64
ROPE_THETA = 10000.0
NCORES = 8
HC = H // 2  # heads per core
MC = HC * DH  # 512, per-core projection width

SM = S // 128  # 16 seq chunks
KD = D // 128  # 8 contraction chunks for projections
MD = HC // 2  # 4 head-pair sweeps per j block
JBLK = 512
NJ = S // JBLK  # 4
TG = 2  # k-transpose group size

F32 = mybir.dt.float32
BF = mybir.dt.bfloat16

# emission cost estimates (ns) for the build-time scheduler
EST_PROJ = 1800.0
EST_KPASS = 2800.0  # proj + rope serialization through the prod ring
EST_VPASS = 2200.0
EST_ITER_PE = 700.0
EST_ITER_ACT = 1045.0
EST_SC = 450.0
BACKLOG_LO = 6000.0  # emit sweep iters while ACT backlog below this


def build_nc():
    nc = bacc.Bacc("TRN2", target_bir_lowering=False, debug=False, num_devices=1)

    hT = nc.dram_tensor("hT", [SM, 128, D], BF, kind="ExternalInput").ap()
    wq = nc.dram_tensor("wq", [128, KD * MC], BF, kind="ExternalInput").ap()
    wk = nc.dram_tensor("wk", [128, KD * MC], BF, kind="ExternalInput").ap()
    wv = nc.dram_tensor("wv", [128, KD * MC], BF, kind="ExternalInput").ap()
    wo = nc.dram_tensor("wo", [128, MD * D], BF, kind="ExternalInput").ap()
    cosq = nc.dram_tensor("cosq", [128, SM * DH], F32, kind="ExternalInput").ap()
    sinq = nc.dram_tensor("sinq", [128, SM * DH], F32, kind="ExternalInput").ap()
    identb = nc.dram_tensor("identb", [128, 128], BF, kind="ExternalInput").ap()
    # out[ph]: contribution of head-pairs 2ph..2ph+1 (summed host-side)
    out = nc.dram_tensor("out", [2, S, D], F32, kind="ExternalOutput").ap()

    with tile.TileContext(nc) as tc:
        _build_body(nc, tc, hT, wq, wk, wv, wo, cosq, sinq, identb, out)
    nc.compile()
    return nc


def _build_body(nc, tc, hT, wq, wk, wv, wo, cosq, sinq, identb, out):
    from contextlib import ExitStack

    Exp = mybir.ActivationFunctionType.Exp

    with ExitStack() as ctx:
        # ---- persistent SBUF ----
        persist = ctx.enter_context(tc.tile_pool(name="persist", bufs=1))
        kt = [persist.tile([128, S], BF, tag=f"kt{d}", name=f"kt{d}") for d in range(MD)]
        # v1[p, m*520 + h*65 + e]: e<64 -> V dims, e==64 -> ones (softmax denom)
        v1 = persist.tile([128, SM * (HC * 65)], BF, tag="v1", name="v1")
        wq_t = persist.tile([128, KD * MC], BF, tag="w_q", name="w_q")
        wk_t = persist.tile([128, KD * MC], BF, tag="w_k", name="w_k")
        wv_t = persist.tile([128, KD * MC], BF, tag="w_v", name="w_v")
        wo_t = persist.tile([128, MD * D], BF, tag="wo", name="wo_t")
        cos_t = persist.tile([128, SM * DH], F32, tag="cos", name="cos_t")
        sin_t = persist.tile([128, SM * DH], F32, tag="sin", name="sin_t")
        identb_t = persist.tile([128, 128], BF, tag="identb", name="identb_t")

        # ---- working SBUF pools ----
        hpool = ctx.enter_context(tc.tile_pool(name="hprod", bufs=8))
        hqpool = ctx.enter_context(tc.tile_pool(name="hq", bufs=4))
        rkpool = ctx.enter_context(tc.tile_pool(name="rk", bufs=4))
        kfpool = ctx.enter_context(tc.tile_pool(name="kf", bufs=2))
        rqpool = ctx.enter_context(tc.tile_pool(name="rq", bufs=4))
        qpool = ctx.enter_context(tc.tile_pool(name="qtj", bufs=3))
        tmpp = ctx.enter_context(tc.tile_pool(name="ropetmp", bufs=3))
        expp = ctx.enter_context(tc.tile_pool(name="expp", bufs=9))
        paccp = ctx.enter_context(tc.tile_pool(name="pacc", bufs=1))
        rcpp = ctx.enter_context(tc.tile_pool(name="rcpp", bufs=8))
        attp = ctx.enter_context(tc.tile_pool(name="attp", bufs=3))
        attTp = ctx.enter_context(tc.tile_pool(name="attTp", bufs=3))
        outp = ctx.enter_context(tc.tile_pool(name="outp", bufs=2))

        # ---- PSUM: 4 + 2 + 2 = 8 banks ----
        scp = ctx.enter_context(tc.tile_pool(name="scp", bufs=2, space="PSUM"))
        avp = ctx.enter_context(tc.tile_pool(name="avp", bufs=2, space="PSUM"))
        prodp = ctx.enter_context(tc.tile_pool(name="prodp", bufs=2, space="PSUM"))

        # ---- init: DMAs ordered for earliest first sweep ----
        h_tiles = {}

        def dma_h(m):
            t = hpool.tile([128, D], BF, tag="h", name="h_m")
            nc.sync.dma_start(out=t[:], in_=hT[m])
            h_tiles[m] = t

        half = 4 * MC
        csh = SM * DH // 2
        nc.sync.dma_start(out=wq_t[:, 0:half], in_=wq[:, 0:half])
        nc.sync.dma_start(out=cos_t[:, 0:csh], in_=cosq[:, 0:csh])
        nc.sync.dma_start(out=sin_t[:, 0:csh], in_=sinq[:, 0:csh])
        dma_h(0)
        dma_h(1)
        nc.sync.dma_start(out=wq_t[:, half:], in_=wq[:, half:])
        nc.sync.dma_start(out=wk_t[:, 0:half], in_=wk[:, 0:half])
        nc.sync.dma_start(out=wk_t[:, half:], in_=wk[:, half:])
        nc.sync.dma_start(out=wv_t[:, 0:half], in_=wv[:, 0:half])
        nc.sync.dma_start(out=identb_t[:], in_=identb)
        dma_h(2)
        dma_h(3)
        nc.sync.dma_start(out=cos_t[:, csh:], in_=cosq[:, csh:])
        nc.sync.dma_start(out=sin_t[:, csh:], in_=sinq[:, csh:])
        nc.sync.dma_start(out=wv_t[:, half:], in_=wv[:, half:])
        dma_h(4)
        dma_h(5)
        dma_h(6)
        dma_h(7)
        nc.sync.dma_start(out=wo_t[:], in_=wo)

        # PE warmup (p-state ramp) on dep-free memset tiles
        wu = persist.tile([128, 512], F32, tag="wu", name="wu")
        nc.vector.memset(wu[:], 0.0)
        wups = prodp.tile([128, 512], F32, tag="prod", name="wups")
        for i in range(2):
            nc.tensor.matmul(
                wups[:], wu[:, 0:128], wu[:], start=(i == 0), stop=(i == 1)
            )

        # ones columns of v1 (Pool memset + DVE broadcast); warm the exp table
        oneb = persist.tile([128, 1], BF, tag="oneb", name="oneb")
        nc.gpsimd.memset(oneb[:], 1.0)
        nc.vector.tensor_copy(
            v1[:].rearrange("p (m h e) -> p m h e", m=SM, h=HC)[:, :, :, 64:65],
            oneb[:, None, None, 0:1].broadcast_to([128, SM, HC, 1]),
        )
        warm = persist.tile([1, 16], F32, tag="warm", name="warm")
        nc.vector.memset(warm[:], 0.0)
        nc.scalar.activation(warm[:], warm[:], Exp)

        # ---- shared helpers ----
        def rope(src, r, m, eng):
            """Natural-layout RoPE: src [128 seq, HC*DH] -> r bf16.
            eng=nc.vector reads PSUM directly (low latency, K path);
            eng=nc.gpsimd needs an SBUF source (Q path, off critical path)."""
            cos_m = cos_t[:, None, m * DH : (m + 1) * DH]
            sin_m = sin_t[:, m * DH : (m + 1) * DH]
            tc_ = tmpp.tile([128, MC], F32, tag="tc", name="tc_")
            ts_ = tmpp.tile([128, MC], F32, tag="ts", name="ts_")
            p3 = src.rearrange("p (h e) -> p h e", h=HC)
            t3 = ts_[:].rearrange("p (h e) -> p h e", h=HC)
            eng.tensor_mul(
                tc_[:].rearrange("p (h e) -> p h e", h=HC),
                p3,
                cos_m.broadcast_to([128, HC, DH]),
            )
            eng.tensor_mul(
                t3[:, :, 0:32],
                p3[:, :, 32:64],
                sin_m[:, None, 0:32].broadcast_to([128, HC, 32]),
            )
            eng.tensor_mul(
                t3[:, :, 32:64],
                p3[:, :, 0:32],
                sin_m[:, None, 32:64].broadcast_to([128, HC, 32]),
            )
            eng.tensor_add(r[:], tc_[:], ts_[:])

        def proj(h_m, w_t, pool=None, tag="prod"):
            ps = (pool or prodp).tile([128, MC], F32, tag=tag, name="ps")
            for kk in range(KD):
                nc.tensor.matmul(
                    ps[:],
                    h_m[:, kk * 128 : (kk + 1) * 128],
                    w_t[:, kk * MC : (kk + 1) * MC],
                    start=(kk == 0),
                    stop=(kk == KD - 1),
                )
            return ps

        # ---- production pieces ----
        rot_k = [None] * TG
        qrot = {j: [None] * 4 for j in range(NJ)}
        qtj = {}

        def k_pass(m):
            ps = proj(h_tiles[m], wk_t)
            r = rkpool.tile([128, MC], BF, tag="rk", name="rk")
            if m < 2:
                # head: keep the DVE queue free for q-ropes; rope on Pool
                kf = kfpool.tile([128, MC], F32, tag="kf", name="kf")
                nc.vector.tensor_copy(kf[:], ps[:])
                rope(kf[:], r, m, nc.gpsimd)
            else:
                rope(ps[:], r, m, nc.vector)
            rot_k[m % TG] = r
            if m % TG == TG - 1:
                m0 = m - (TG - 1)
                W = TG * 128
                for dpair in range(2):
                    tps = prodp.tile([128, 2 * W], BF, tag="prod", name="ktps")
                    for half in range(2):
                        d = dpair * 2 + half
                        for mm in range(TG):
                            nc.tensor.transpose(
                                tps[:, half * W + mm * 128 : half * W + (mm + 1) * 128],
                                rot_k[mm][:, d * 128 : (d + 1) * 128],
                                identb_t[:],
                            )
                    for half in range(2):
                        d = dpair * 2 + half
                        nc.vector.tensor_copy(
                            kt[d][:, m0 * 128 : m0 * 128 + W],
                            tps[:, half * W : (half + 1) * W],
                        )

        def v_pass(m):
            ps = proj(h_tiles[m], wv_t)
            dst = v1[:, m * (HC * 65) : (m + 1) * (HC * 65)].rearrange(
                "p (h e) -> p h e", h=HC
            )[:, :, 0:64]
            nc.vector.tensor_copy(dst, ps[:].rearrange("p (h e) -> p h e", h=HC))
            h_tiles.pop(m)

        def q_pass(j, k):
            m = j * 4 + k
            if j == 0:
                h_m = h_tiles[m]
            else:
                h_m = h_tiles.pop(("q", m))
            # j0's projections borrow the av banks (idle until the first AV)
            # so K production streams through prodp in parallel
            ps = proj(h_m, wq_t, pool=avp if j == 0 else None,
                      tag="av" if j == 0 else "prod")
            r = rqpool.tile([128, MC], BF, tag="rq", name="rq")
            rope(ps[:], r, m, nc.vector)
            qrot[j][k] = r

        def q_dma(j, k):
            m = j * 4 + k
            t = hqpool.tile([128, D], BF, tag="hq", name="hq")
            nc.sync.dma_start(out=t[:], in_=hT[m])
            h_tiles[("q", m)] = t

        def q_trans(j):
            tiles = [
                qpool.tile([128, JBLK], BF, tag=f"qt{d}", name=f"qt{d}")
                for d in range(MD)
            ]
            for dpair in range(2):
                tps = prodp.tile([128, 1024], BF, tag="prod", name="qtps")
                for half in range(2):
                    d = dpair * 2 + half
                    for mm in range(4):
                        nc.tensor.transpose(
                            tps[:, half * 512 + mm * 128 : half * 512 + (mm + 1) * 128],
                            qrot[j][mm][:, d * 128 : (d + 1) * 128],
                            identb_t[:],
                        )
                for half in range(2):
                    d = dpair * 2 + half
                    nc.vector.tensor_copy(
                        tiles[d][:], tps[:, half * 512 : (half + 1) * 512]
                    )
            qrot[j] = [None] * 4
            qtj[j] = tiles

        # ---- sweep machinery ----
        # state per sweep (j,d): next_m, av pair or None, pacc tile, pending AV
        att_state = {}  # j -> [att tiles per qc]
        attT_state = {}  # j -> [attT tiles per kk]

        PENDING_DEPTH = 2  # min AV emission lag behind exp (iters)
        CLOSE_LATENCY = 2600.0  # est ns from close emission to av-bank free

        class Sweep:
            def __init__(self, j, d):
                self.j = j
                self.d = d
                self.next_m = 0
                self.av = None  # (av0, av1) psum pair while segment open
                self.seg_start = True  # next pending AV opens a segment
                self.pacc = None
                self.pending = deque()  # (m, ex) awaiting AV emission
                self.done = False

        def emit_pending_av(sw, stop):
            m, ex = sw.pending.popleft()
            if sw.av is None:
                sw.av = (
                    avp.tile([128, 512], F32, tag="av", name="av0"),
                    avp.tile([128, 512], F32, tag="av", name="av1"),
                )
                sw.seg_start = True
            for hh in range(2):
                h2 = 2 * sw.d + hh
                vs = m * (HC * 65) + h2 * 65
                for qc in range(4):
                    nc.tensor.matmul(
                        sw.av[hh][:, qc * 65 : (qc + 1) * 65],
                        ex[:, hh * JBLK + qc * 128 : hh * JBLK + (qc + 1) * 128],
                        v1[:, vs : vs + 65],
                        start=(sw.seg_start and qc == 0),
                        stop=(stop and qc == 3),
                    )
            sw.seg_start = False

        def emit_iter(sw):
            """One m iteration: sc matmuls + exp; AV of previous iter."""
            m = sw.next_m
            sw.next_m += 1
            d, j = sw.d, sw.j
            sc = scp.tile([128, 2 * JBLK], F32, tag="sc", name="sc")
            nc.tensor.matmul(
                sc[:, 0:JBLK],
                kt[d][0:64, m * 128 : (m + 1) * 128],
                qtj[j][d][0:64, :],
                start=True,
                stop=True,
            )
            nc.tensor.matmul(
                sc[:, JBLK : 2 * JBLK],
                kt[d][64:128, m * 128 : (m + 1) * 128],
                qtj[j][d][64:128, :],
                start=True,
                stop=True,
            )
            drain_pending(sw)
            ex = expp.tile([128, 2 * JBLK], BF, tag="ex", name="ex")
            nc.scalar.activation(ex[:], sc[:], Exp)
            sw.pending.append((m, ex))

        av_free_est = [0.0]

        def drain_pending(sw):
            """Emit one deferred AV unless a new segment would have to wait
            for the av pair to drain (let exps run ahead instead). Hard cap:
            an exp's ex-tile allocation WARs against the AV 10 allocations
            back (expp ring) — if AVs lag more than ring-2, the sem graph
            deadlocks (exp waits AV, AV behind a blocked sc, sc waits exp)."""
            if len(sw.pending) <= PENDING_DEPTH:
                return
            if (
                len(sw.pending) < 7
                and pe_t < av_free_est[0]
                and sw.av is None
            ):
                return
            emit_pending_av(sw, stop=False)

        def close_segment(sw):
            """Evict/accumulate the open segment (or finish the sweep)."""
            final = sw.next_m == SM and bool(sw.pending)
            while sw.pending:
                emit_pending_av(sw, stop=len(sw.pending) == 1)
            av_free_est[0] = pe_t + CLOSE_LATENCY
            if sw.av is None:
                return
            av0, av1 = sw.av
            sw.av = None
            j, d = sw.j, sw.d
            if final and sw.pacc is None:
                # single full segment: normalize straight from psum (DVE)
                post = all(s.done for s in sweeps if s is not sw)
                norm(j, d, (av0[:, 0:260], av1[:, 0:260]), sbuf_src=False,
                     post_exp=post)
                sw.done = True
                return
            if sw.pacc is None:
                sw.pacc = paccp.tile(
                    [128, 520], F32, tag=f"pacc{j}{d}", name=f"pacc{j}{d}"
                )
                nc.vector.tensor_copy(sw.pacc[:, 0:260], av0[:, 0:260])
                nc.vector.tensor_copy(sw.pacc[:, 260:520], av1[:, 0:260])
            else:
                nc.vector.tensor_add(sw.pacc[:, 0:260], sw.pacc[:, 0:260], av0[:, 0:260])
                nc.vector.tensor_add(sw.pacc[:, 260:520], sw.pacc[:, 260:520], av1[:, 0:260])
            if final:
                post = all(s.done for s in sweeps if s is not sw)
                norm(j, d, (sw.pacc[:, 0:260], sw.pacc[:, 260:520]), sbuf_src=True,
                     post_exp=post)
                sw.done = True

        def norm(j, d, halves, sbuf_src=False, post_exp=False):
            if j not in att_state:
                att_state[j] = [
                    attp.tile([128, JBLK], BF, tag=f"att{qc}", name=f"att{qc}")
                    for qc in range(4)
                ]
            att_j = att_state[j]
            # Pool can't read PSUM; only pacc (SBUF) sources may use it
            mul_eng = nc.gpsimd if sbuf_src else nc.vector
            Copy = mybir.ActivationFunctionType.Copy
            for hh in range(2):
                src = halves[hh]
                h2 = 2 * d + hh
                for qc in range(4):
                    s0 = qc * 65
                    rcp = rcpp.tile([128, 1], F32, tag="rcp", name="rcp")
                    nc.vector.reciprocal(rcp[:], src[:, s0 + 64 : s0 + 65])
                    dst = att_j[qc][:, h2 * 64 : (h2 + 1) * 64]
                    mul_eng.tensor_scalar_mul(dst, src[:, s0 : s0 + 64], rcp[:])

        # ---- aux pieces (attT / wo) ----
        def attT_piece(j, pair, pool=None):
            pool = pool or prodp
            tag = "prod" if pool is prodp else "sc"
            att_j = att_state[j]
            attT_j = attT_state.setdefault(j, [None] * MD)
            tps = pool.tile([128, 1024], BF, tag=tag, name="atps")
            for half in range(2):
                kk = pair * 2 + half
                for qc in range(4):
                    nc.tensor.transpose(
                        tps[:, half * 512 + qc * 128 : half * 512 + (qc + 1) * 128],
                        att_j[qc][:, kk * 128 : (kk + 1) * 128],
                        identb_t[:],
                    )
            for half in range(2):
                kk = pair * 2 + half
                t = attTp.tile([128, JBLK], BF, tag=f"attT{kk}", name="attT")
                nc.vector.tensor_copy(t[:], tps[:, half * 512 : (half + 1) * 512])
                attT_j[kk] = t

        def wo_half(j, qc, pair):
            """Head-pairs 2*pair..2*pair+1 contribution to out[pair] rows."""
            attT_j = attT_state[j]
            wops = [
                prodp.tile([128, 512], F32, tag="prod", name=f"wops{nb}")
                for nb in range(2)
            ]
            for nb in range(2):
                for kk in (2 * pair, 2 * pair + 1):
                    nc.tensor.matmul(
                        wops[nb][:],
                        attT_j[kk][:, qc * 128 : (qc + 1) * 128],
                        wo_t[:, kk * D + nb * 512 : kk * D + nb * 512 + 512],
                        start=(kk == 2 * pair),
                        stop=(kk == 2 * pair + 1),
                    )
            ot = outp.tile([128, D], F32, tag="ot", name="ot")
            mrow = j * JBLK + qc * 128
            if all(s.done for s in sweeps):
                # tail: chain each half's DMA behind its eviction
                for nb in range(2):
                    nc.vector.tensor_copy(ot[:, nb * 512 : (nb + 1) * 512], wops[nb][:])
                    nc.sync.dma_start(
                        out=out[pair, mrow : mrow + 128, nb * 512 : (nb + 1) * 512],
                        in_=ot[:, nb * 512 : (nb + 1) * 512],
                    )
            else:
                nc.vector.tensor_copy(ot[:, 0:512], wops[0][:])
                nc.vector.tensor_copy(ot[:, 512:1024], wops[1][:])
                nc.sync.dma_start(out=out[pair, mrow : mrow + 128, :], in_=ot[:])

        # ---- build-time scheduler ----
        # production queue: (kind, args, pe_cost)
        prod_q = deque()
        prod_q.append(("qp", (0, 0), EST_PROJ))
        prod_q.append(("k", (0,), EST_KPASS))
        prod_q.append(("qp", (0, 1), EST_PROJ))
        prod_q.append(("k", (1,), EST_KPASS + 1100.0))
        prod_q.append(("qp", (0, 2), EST_PROJ))
        prod_q.append(("qp", (0, 3), EST_PROJ))
        prod_q.append(("qt", (0,), 900.0))
        prod_q.append(("k", (2,), EST_KPASS))
        prod_q.append(("v", (0,), EST_VPASS))
        prod_q.append(("k", (3,), EST_KPASS + 1100.0))
        prod_q.append(("v", (1,), EST_VPASS))
        prod_q.append(("k", (4,), EST_KPASS))
        prod_q.append(("k", (5,), EST_KPASS + 1100.0))
        prod_q.append(("k", (6,), EST_KPASS))
        prod_q.append(("k", (7,), EST_KPASS + 1100.0))
        for m in range(2, SM):
            if m + 6 < SM:
                prod_q.append(("hdma", (m + 6,), 0.0))
            if m >= 8:
                # k staggered ahead so kt groups land early
                kcost = EST_KPASS + (1100.0 if m % TG == TG - 1 else 0.0)
                prod_q.append(("k", (m,), kcost))
            if 3 <= m <= 6:
                prod_q.append(("qdma", (1, m - 3), 0.0))
            if 5 <= m <= 8:
                prod_q.append(("qp", (1, m - 5), EST_PROJ))
            if 7 <= m <= 10:
                prod_q.append(("qdma", (2, m - 7), 0.0))
            if 9 <= m <= 12:
                prod_q.append(("qp", (2, m - 9), EST_PROJ))
            prod_q.append(("v", (m,), EST_VPASS))
            if m == 8:
                prod_q.append(("qt", (1,), 900.0))
            if m == 12:
                prod_q.append(("qt", (2,), 900.0))
        for j in (3,):
            prod_q.append(("qdma", (j, 0), 0.0))
            prod_q.append(("qdma", (j, 1), 0.0))
            for k in range(4):
                if k + 2 < 4:
                    prod_q.append(("qdma", (j, k + 2), 0.0))
                prod_q.append(("qp", (j, k), EST_PROJ))
            prod_q.append(("qt", (j,), 900.0))

        produced_k = set()
        produced_v = set()
        qt_ready = set()

        def run_prod(item):
            kind, args, _ = item
            if kind == "k":
                k_pass(*args)
                m = args[0]
                if m % TG == TG - 1:
                    for mm in range(m - TG + 1, m + 1):
                        produced_k.add(mm)
            elif kind == "v":
                v_pass(*args)
                produced_v.add(args[0])
            elif kind == "qp":
                q_pass(*args)
            elif kind == "qt":
                q_trans(*args)
                qt_ready.add(args[0])
            elif kind == "hdma":
                dma_h(*args)
            elif kind == "qdma":
                j, k = args
                q_dma(j, k)

        sweeps = [Sweep(j, d) for j in range(NJ) for d in range(MD)]
        aux_q = deque()  # (kind, args, pe_cost), dependency-ready aux work
        norm_count = {}  # j -> number of normed sweeps

        def on_sweep_done(sw):
            j = sw.j
            norm_count[j] = norm_count.get(j, 0) + 1
            dlist = [s.d for s in sweeps if s.j == j and s.done]
            for pair in range(2):
                if (
                    2 * pair in dlist
                    and 2 * pair + 1 in dlist
                    and (f"attT{pair}", j) not in emitted_aux
                ):
                    emitted_aux.add((f"attT{pair}", j))
                    # last attT of the kernel: sc ring is idle, borrow it
                    borrow_sc = j == NJ - 1 and pair == 1
                    aux_q.append(("attT", (j, pair, borrow_sc), 500.0))

        emitted_aux = set()

        def run_aux(item):
            kind, args, _ = item
            if kind == "attT":
                j, pair, borrow_sc = args
                attT_piece(j, pair, pool=scp if borrow_sc else prodp)
                for qc in range(4):
                    aux_q.append(("woh", (j, qc, pair), 950.0))
            elif kind == "woh":
                wo_half(*args)

        def sweep_ready(sw):
            return (
                not sw.done
                and sw.next_m < SM
                and sw.j in qt_ready
                and sw.next_m in produced_k
                and sw.next_m in produced_v
            )

        pe_t = 0.0
        act_t = 0.0
        active = None

        def emit_one_iter(sw):
            nonlocal pe_t, act_t, active
            if active is not None and active is not sw:
                close_segment(active)
                pe_t += 100.0
            active = sw
            emit_iter(sw)
            pe_t += EST_ITER_PE
            act_t = max(act_t + EST_ITER_ACT, pe_t + EST_SC + EST_ITER_ACT)

        def pick_sweep():
            if active is not None and sweep_ready(active):
                return active
            ready = [sw for sw in sweeps if sweep_ready(sw)]
            if not ready:
                return None

            def avail(sw):
                hi = sw.next_m
                while hi < SM and hi in produced_k and hi in produced_v:
                    hi += 1
                return hi - sw.next_m

            # finish low-j blocks first (staggers attT/wo aux); among same j
            # prefer the sweep with most available chunks (fewer switches)
            return min(ready, key=lambda sw: (sw.j, -avail(sw)))

        while True:
            remaining = [sw for sw in sweeps if not sw.done]
            if not remaining and not prod_q and not aux_q:
                break
            backlog = act_t - pe_t
            cand = pick_sweep() if backlog < BACKLOG_LO else None
            if cand is None and not prod_q and not aux_q:
                cand = pick_sweep()  # ACT-bound tail: keep sweeping
            if cand is not None:
                emit_one_iter(cand)
                if cand.next_m == SM:
                    close_segment(cand)
                    if cand is active:
                        active = None
                    on_sweep_done(cand)
                continue
            if prod_q:
                item = prod_q.popleft()
                run_prod(item)
                pe_t += item[2]
                continue
            if aux_q:
                item = aux_q.popleft()
                run_aux(item)
                pe_t += item[2]
                continue
            raise RuntimeError(
                f"scheduler deadlock: {[(sw.j, sw.d, sw.next_m) for sw in remaining]}"
            )
        assert all(sw.done for sw in sweeps)


def _wtile(w, dtype):
    """[K*128, N] -> [128, K*N] with tile[p, kk*N+c] = w[128*kk+p, c]."""
    kchunks = w.shape[0] // 128
    return np.ascontiguousarray(
        w.reshape(kchunks, 128, w.shape[1])
        .transpose(1, 0, 2)
        .reshape(128, kchunks * w.shape[1])
        .astype(dtype)
    )


def prep_core_inputs(positions, hidden_states, wq, wk, wv, wo):
    """Host-side sharding/pre-tiling. Returns list of 8 in_maps."""
    import ml_dtypes

    bf16 = ml_dtypes.bfloat16
    pos = np.asarray(positions).astype(np.float32)
    inv_freq = 1.0 / (ROPE_THETA ** (np.arange(0, DH, 2, dtype=np.float32) / DH))
    ang = pos[:, None] * inv_freq[None, :]  # [S, 32]
    cos_half = np.cos(ang).astype(np.float32)
    sin_half = np.sin(ang).astype(np.float32)
    cos_full = np.concatenate([cos_half, cos_half], axis=1)  # [S, 64]
    sin_signed = np.concatenate([-sin_half, sin_half], axis=1)  # [S, 64]
    cos_tiled = np.ascontiguousarray(
        cos_full.reshape(SM, 128, DH).transpose(1, 0, 2).reshape(128, SM * DH)
    )
    sin_tiled = np.ascontiguousarray(
        sin_signed.reshape(SM, 128, DH).transpose(1, 0, 2).reshape(128, SM * DH)
    )
    identb = np.eye(128, dtype=np.float32).astype(bf16)

    hs = np.asarray(hidden_states, dtype=np.float32)
    # hT[m, p, kk*128+cc] = hidden[b, 128m+cc, 128kk+p]
    hT_all = []
    for b in range(B):
        t = hs[b].reshape(SM, 128, KD, 128).transpose(0, 3, 2, 1)  # (m, p, kk, cc)
        hT_all.append(np.ascontiguousarray(t.reshape(SM, 128, D)).astype(bf16))

    in_maps = []
    for c in range(NCORES):
        b = c // 2
        g = c % 2
        sl = slice(g * MC, (g + 1) * MC)
        in_maps.append(
            {
                "hT": hT_all[b],
                "wq": _wtile(np.asarray(wq, np.float32)[:, sl], bf16),
                "wk": _wtile(np.asarray(wk, np.float32)[:, sl], bf16),
                "wv": _wtile(np.asarray(wv, np.float32)[:, sl], bf16),
                "wo": _wtile(np.asarray(wo, np.float32)[sl, :], bf16),
                "cosq": cos_tiled,
                "sinq": sin_tiled,
                "identb": identb,
            }
        )
    return in_maps


_NC_CACHE = {}


def get_nc():
    if "nc" not in _NC_CACHE:
        _NC_CACHE["nc"] = build_nc()
    return _NC_CACHE["nc"]


def kernel(positions, hidden_states, wq, wk, wv, wo):
    in_dtype = np.asarray(hidden_states).dtype
    in_maps = prep_core_inputs(positions, hidden_states, wq, wk, wv, wo)
    nc = get_nc()
    res = bass_utils.run_bass_kernel_spmd(nc, in_maps, core_ids=list(range(NCORES)))
    outs = np.empty((B, S, D), dtype=np.float32)
    for b in range(B):
        o0 = res.results[2 * b]["out"]
        o1 = res.results[2 * b + 1]["out"]
        outs[b] = (o0[0] + o0[1]) + (o1[0] + o1[1])
    return outs.astype(in_dtype, copy=False)


# revision 101
# speedup vs baseline: 1.1907x; 1.0002x over previous
"""Trainium2 Bass kernel for Chronos2Attention (B=4, S=2048, D=1024, H=16, Dh=64).

Sharding: 8 cores = 4 batches x 2 head-groups. Core c handles batch c//2 and
heads 8*(c%2) .. 8*(c%2)+7 (wq/wk/wv column-sharded, wo row-sharded); host sums
the two partial [S, D] outputs per batch at gather time.

v4 design (vs v3): fully fused production/attention pipeline.
  - All PE-side tensors bf16 (h, weights, roped q/k, kt, v, attn) -> every
    matmul/transpose runs at 1 cyc/row; halved DMA.
  - K/V/Q production streams through a 2-buf [128,512]f32 PSUM "prod" pool
    just-in-time; attention sweeps consume chunks as they appear. j0's Q
    projections borrow the (still idle) av banks so K can start in parallel;
    head k-ropes run on Pool to keep the DVE queue short.
  - Sweeps (j = 512-query block, d = head pair) are SEGMENTED: AV accumulates
    in a single PSUM bank-pair per segment (2 heads x 4qc x 65 cols with the
    ones/denominator column), partial segments accumulated into SBUF f32
    "pacc" tiles so 2 av banks suffice while many sweeps are in flight
    against a limited set of produced chunks.
  - AV matmuls are emitted LAZILY (deque per sweep, depth-capped by the
    ex-tile ring) so exps can run ahead across av-bank handoffs; the stop
    flag lands on the true last matmul of each segment.
  - Emission is availability-driven: a tiny cost model interleaves production
    passes, sweep iterations (sc->exp->AV), and aux work (norm/attT/wo/q-proj)
    to keep ACT (exp, the ~266us floor) saturated and PE dense. Engines:
    ACT = exp only (anything else deadlock-prone via ring WARs); DVE = ropes,
    psum evictions, pacc, norms-from-psum; Pool = head ropes + norms-from-pacc.
  - Wo is computed in head-pair halves into out[2, S, D] (host sums 4 partial
    outputs per batch) so each j block's Wo work drains early; the final
    block's last attT borrows the then-idle sc ring.
  - PSUM budget: sc ring 2x[128,1024] (4 banks) + av 2x[128,512] (2) +
    prod 2x[128,512] (2) = 8 banks.
"""

from collections import deque

import numpy as np

import concourse.bacc as bacc
import concourse.mybir as mybir
import concourse.tile as tile
from concourse import bass_utils

# Problem shapes (hardcoded per spec)
B = 4
S = 2048
D = 1024
H = 16
DH = 64
ROPE_THETA = 10000.0
NCORES = 8
HC = H // 2  # heads per core
MC = HC * DH  # 512, per-core projection width

SM = S // 128  # 16 seq chunks
KD = D // 128  # 8 contraction chunks for projections
MD = HC // 2  # 4 head-pair sweeps per j block
JBLK = 512
NJ = S // JBLK  # 4
TG = 2  # k-transpose group size

F32 = mybir.dt.float32
BF = mybir.dt.bfloat16

# emission cost estimates (ns) for the build-time scheduler
EST_PROJ = 1800.0
EST_KPASS = 2800.0  # proj + rope serialization through the prod ring
EST_VPASS = 2200.0
EST_ITER_PE = 700.0
EST_ITER_ACT = 1045.0
EST_SC = 450.0
BACKLOG_LO = 6000.0  # emit sweep iters while ACT backlog below this


def build_nc():
    nc = bacc.Bacc("TRN2", target_bir_lowering=False, debug=False, num_devices=1)

    hT = nc.dram_tensor("hT", [SM, 128, D], BF, kind="ExternalInput").ap()
    wq = nc.dram_tensor("wq", [128, KD * MC], BF, kind="ExternalInput").ap()
    wk = nc.dram_tensor("wk", [128, KD * MC], BF, kind="ExternalInput").ap()
    wv = nc.dram_tensor("wv", [128, KD * MC], BF, kind="ExternalInput").ap()
    wo = nc.dram_tensor("wo", [128, MD * D], BF, kind="ExternalInput").ap()
    cosq = nc.dram_tensor("cosq", [128, SM * DH], F32, kind="ExternalInput").ap()
    sinq = nc.dram_tensor("sinq", [128, SM * DH], F32, kind="ExternalInput").ap()
    identb = nc.dram_tensor("identb", [128, 128], BF, kind="ExternalInput").ap()
    # out[ph]: contribution of head-pairs 2ph..2ph+1 (summed host-side)
    out = nc.dram_tensor("out", [2, S, D], F32, kind="ExternalOutput").ap()

    with tile.TileContext(nc) as tc:
        _build_body(nc, tc, hT, wq, wk, wv, wo, cosq, sinq, identb, out)
    nc.compile()
    return nc


def _build_body(nc, tc, hT, wq, wk, wv, wo, cosq, sinq, identb, out):
    from contextlib import ExitStack

    Exp = mybir.ActivationFunctionType.Exp

    with ExitStack() as ctx:
        # ---- persistent SBUF ----
        persist = ctx.enter_context(tc.tile_pool(name="persist", bufs=1))
        kt = [persist.tile([128, S], BF, tag=f"kt{d}", name=f"kt{d}") for d in range(MD)]
        # v1[p, m*520 + h*65 + e]: e<64 -> V dims, e==64 -> ones (softmax denom)
        v1 = persist.tile([128, SM * (HC * 65)], BF, tag="v1", name="v1")
        wq_t = persist.tile([128, KD * MC], BF, tag="w_q", name="w_q")
        wk_t = persist.tile([128, KD * MC], BF, tag="w_k", name="w_k")
        wv_t = persist.tile([128, KD * MC], BF, tag="w_v", name="w_v")
        wo_t = persist.tile([128, MD * D], BF, tag="wo", name="wo_t")
        cos_t = persist.tile([128, SM * DH], F32, tag="cos", name="cos_t")
        sin_t = persist.tile([128, SM * DH], F32, tag="sin", name="sin_t")
        identb_t = persist.tile([128, 128], BF, tag="identb", name="identb_t")

        # ---- working SBUF pools ----
        hpool = ctx.enter_context(tc.tile_pool(name="hprod", bufs=8))
        hqpool = ctx.enter_context(tc.tile_pool(name="hq", bufs=4))
        rkpool = ctx.enter_context(tc.tile_pool(name="rk", bufs=4))
        kfpool = ctx.enter_context(tc.tile_pool(name="kf", bufs=2))
        rqpool = ctx.enter_context(tc.tile_pool(name="rq", bufs=4))
        qpool = ctx.enter_context(tc.tile_pool(name="qtj", bufs=3))
        tmpp = ctx.enter_context(tc.tile_pool(name="ropetmp", bufs=3))
        expp = ctx.enter_context(tc.tile_pool(name="expp", bufs=9))
        paccp = ctx.enter_context(tc.tile_pool(name="pacc", bufs=1))
        rcpp = ctx.enter_context(tc.tile_pool(name="rcpp", bufs=8))
        attp = ctx.enter_context(tc.tile_pool(name="attp", bufs=3))
        attTp = ctx.enter_context(tc.tile_pool(name="attTp", bufs=3))
        outp = ctx.enter_context(tc.tile_pool(name="outp", bufs=2))

        # ---- PSUM: 4 + 2 + 2 = 8 banks ----
        scp = ctx.enter_context(tc.tile_pool(name="scp", bufs=2, space="PSUM"))
        avp = ctx.enter_context(tc.tile_pool(name="avp", bufs=2, space="PSUM"))
        prodp = ctx.enter_context(tc.tile_pool(name="prodp", bufs=2, space="PSUM"))

        # ---- init: DMAs ordered for earliest first sweep ----
        h_tiles = {}

        def dma_h(m):
            t = hpool.tile([128, D], BF, tag="h", name="h_m")
            nc.sync.dma_start(out=t[:], in_=hT[m])
            h_tiles[m] = t

        half = 4 * MC
        csh = SM * DH // 2
        nc.sync.dma_start(out=wq_t[:, 0:half], in_=wq[:, 0:half])
        nc.sync.dma_start(out=cos_t[:, 0:csh], in_=cosq[:, 0:csh])
        nc.sync.dma_start(out=sin_t[:, 0:csh], in_=sinq[:, 0:csh])
        dma_h(0)
        dma_h(1)
        nc.sync.dma_start(out=wq_t[:, half:], in_=wq[:, half:])
        nc.sync.dma_start(out=wk_t[:, 0:half], in_=wk[:, 0:half])
        nc.sync.dma_start(out=wk_t[:, half:], in_=wk[:, half:])
        nc.sync.dma_start(out=wv_t[:, 0:half], in_=wv[:, 0:half])
        nc.sync.dma_start(out=identb_t[:], in_=identb)
        dma_h(2)
        dma_h(3)
        nc.sync.dma_start(out=cos_t[:, csh:], in_=cosq[:, csh:])
        nc.sync.dma_start(out=sin_t[:, csh:], in_=sinq[:, csh:])
        nc.sync.dma_start(out=wv_t[:, half:], in_=wv[:, half:])
        dma_h(4)
        dma_h(5)
        dma_h(6)
        dma_h(7)
        nc.sync.dma_start(out=wo_t[:], in_=wo)

        # PE warmup (p-state ramp) on dep-free memset tiles
        wu = persist.tile([128, 512], F32, tag="wu", name="wu")
        nc.vector.memset(wu[:], 0.0)
        wups = prodp.tile([128, 512], F32, tag="prod", name="wups")
        for i in range(2):
            nc.tensor.matmul(
                wups[:], wu[:, 0:128], wu[:], start=(i == 0), stop=(i == 1)
            )

        # ones columns of v1 (Pool memset + DVE broadcast); warm the exp table
        oneb = persist.tile([128, 1], BF, tag="oneb", name="oneb")
        nc.gpsimd.memset(oneb[:], 1.0)
        nc.vector.tensor_copy(
            v1[:].rearrange("p (m h e) -> p m h e", m=SM, h=HC)[:, :, :, 64:65],
            oneb[:, None, None, 0:1].broadcast_to([128, SM, HC, 1]),
        )
        warm = persist.tile([1, 16], F32, tag="warm", name="warm")
        nc.vector.memset(warm[:], 0.0)
        nc.scalar.activation(warm[:], warm[:], Exp)

        # ---- shared helpers ----
        def rope(src, r, m, eng):
            """Natural-layout RoPE: src [128 seq, HC*DH] -> r bf16.
            eng=nc.vector reads PSUM directly (low latency, K path);
            eng=nc.gpsimd needs an SBUF source (Q path, off critical path)."""
            cos_m = cos_t[:, None, m * DH : (m + 1) * DH]
            sin_m = sin_t[:, m * DH : (m + 1) * DH]
            tc_ = tmpp.tile([128, MC], F32, tag="tc", name="tc_")
            ts_ = tmpp.tile([128, MC], F32, tag="ts", name="ts_")
            p3 = src.rearrange("p (h e) -> p h e", h=HC)
            t3 = ts_[:].rearrange("p (h e) -> p h e", h=HC)
            eng.tensor_mul(
                tc_[:].rearrange("p (h e) -> p h e", h=HC),
                p3,
                cos_m.broadcast_to([128, HC, DH]),
            )
            eng.tensor_mul(
                t3[:, :, 0:32],
                p3[:, :, 32:64],
                sin_m[:, None, 0:32].broadcast_to([128, HC, 32]),
            )
            eng.tensor_mul(
                t3[:, :, 32:64],
                p3[:, :, 0:32],
                sin_m[:, None, 32:64].broadcast_to([128, HC, 32]),
            )
            eng.tensor_add(r[:], tc_[:], ts_[:])

        def proj(h_m, w_t, pool=None, tag="prod"):
            ps = (pool or prodp).tile([128, MC], F32, tag=tag, name="ps")
            for kk in range(KD):
                nc.tensor.matmul(
                    ps[:],
                    h_m[:, kk * 128 : (kk + 1) * 128],
                    w_t[:, kk * MC : (kk + 1) * MC],
                    start=(kk == 0),
                    stop=(kk == KD - 1),
                )
            return ps

        # ---- production pieces ----
        rot_k = [None] * TG
        qrot = {j: [None] * 4 for j in range(NJ)}
        qtj = {}

        def k_pass(m):
            ps = proj(h_tiles[m], wk_t)
            r = rkpool.tile([128, MC], BF, tag="rk", name="rk")
            if m < 2:
                # head: keep the DVE queue free for q-ropes; rope on Pool
                kf = kfpool.tile([128, MC], F32, tag="kf", name="kf")
                nc.vector.tensor_copy(kf[:], ps[:])
                rope(kf[:], r, m, nc.gpsimd)
            else:
                rope(ps[:], r, m, nc.vector)
            rot_k[m % TG] = r
            if m % TG == TG - 1:
                m0 = m - (TG - 1)
                W = TG * 128
                for dpair in range(2):
                    tps = prodp.tile([128, 2 * W], BF, tag="prod", name="ktps")
                    for half in range(2):
                        d = dpair * 2 + half
                        for mm in range(TG):
                            nc.tensor.transpose(
                                tps[:, half * W + mm * 128 : half * W + (mm + 1) * 128],
                                rot_k[mm][:, d * 128 : (d + 1) * 128],
                                identb_t[:],
                            )
                    for half in range(2):
                        d = dpair * 2 + half
                        nc.vector.tensor_copy(
                            kt[d][:, m0 * 128 : m0 * 128 + W],
                            tps[:, half * W : (half + 1) * W],
                        )

        def v_pass(m):
            ps = proj(h_tiles[m], wv_t)
            dst = v1[:, m * (HC * 65) : (m + 1) * (HC * 65)].rearrange(
                "p (h e) -> p h e", h=HC
            )[:, :, 0:64]
            nc.vector.tensor_copy(dst, ps[:].rearrange("p (h e) -> p h e", h=HC))
            h_tiles.pop(m)

        def q_pass(j, k):
            m = j * 4 + k
            if j == 0:
                h_m = h_tiles[m]
            else:
                h_m = h_tiles.pop(("q", m))
            # j0's projections borrow the av banks (idle until the first AV)
            # so K production streams through prodp in parallel
            ps = proj(h_m, wq_t, pool=avp if j == 0 else None,
                      tag="av" if j == 0 else "prod")
            r = rqpool.tile([128, MC], BF, tag="rq", name="rq")
            rope(ps[:], r, m, nc.vector)
            qrot[j][k] = r

        def q_dma(j, k):
            m = j * 4 + k
            t = hqpool.tile([128, D], BF, tag="hq", name="hq")
            nc.sync.dma_start(out=t[:], in_=hT[m])
            h_tiles[("q", m)] = t

        def q_trans(j):
            tiles = [
                qpool.tile([128, JBLK], BF, tag=f"qt{d}", name=f"qt{d}")
                for d in range(MD)
            ]
            for dpair in range(2):
                tps = prodp.tile([128, 1024], BF, tag="prod", name="qtps")
                for half in range(2):
                    d = dpair * 2 + half
                    for mm in range(4):
                        nc.tensor.transpose(
                            tps[:, half * 512 + mm * 128 : half * 512 + (mm + 1) * 128],
                            qrot[j][mm][:, d * 128 : (d + 1) * 128],
                            identb_t[:],
                        )
                for half in range(2):
                    d = dpair * 2 + half
                    nc.vector.tensor_copy(
                        tiles[d][:], tps[:, half * 512 : (half + 1) * 512]
                    )
            qrot[j] = [None] * 4
            qtj[j] = tiles

        # ---- sweep machinery ----
        # state per sweep (j,d): next_m, av pair or None, pacc tile, pending AV
        att_state = {}  # j -> [att tiles per qc]
        attT_state = {}  # j -> [attT tiles per kk]

        PENDING_DEPTH = 2  # min AV emission lag behind exp (iters)
        CLOSE_LATENCY = 2600.0  # est ns from close emission to av-bank free

        class Sweep:
            def __init__(self, j, d):
                self.j = j
                self.d = d
                self.next_m = 0
                self.av = None  # (av0, av1) psum pair while segment open
                self.seg_start = True  # next pending AV opens a segment
                self.pacc = None
                self.pending = deque()  # (m, ex) awaiting AV emission
                self.done = False

        def emit_pending_av(sw, stop):
            m, ex = sw.pending.popleft()
            if sw.av is None:
                sw.av = (
                    avp.tile([128, 512], F32, tag="av", name="av0"),
                    avp.tile([128, 512], F32, tag="av", name="av1"),
                )
                sw.seg_start = True
            for hh in range(2):
                h2 = 2 * sw.d + hh
                vs = m * (HC * 65) + h2 * 65
                for qc in range(4):
                    nc.tensor.matmul(
                        sw.av[hh][:, qc * 65 : (qc + 1) * 65],
                        ex[:, hh * JBLK + qc * 128 : hh * JBLK + (qc + 1) * 128],
                        v1[:, vs : vs + 65],
                        start=(sw.seg_start and qc == 0),
                        stop=(stop and qc == 3),
                    )
            sw.seg_start = False

        def emit_iter(sw):
            """One m iteration: sc matmuls + exp; AV of previous iter."""
            m = sw.next_m
            sw.next_m += 1
            d, j = sw.d, sw.j
            sc = scp.tile([128, 2 * JBLK], F32, tag="sc", name="sc")
            nc.tensor.matmul(
                sc[:, 0:JBLK],
                kt[d][0:64, m * 128 : (m + 1) * 128],
                qtj[j][d][0:64, :],
                start=True,
                stop=True,
            )
            nc.tensor.matmul(
                sc[:, JBLK : 2 * JBLK],
                kt[d][64:128, m * 128 : (m + 1) * 128],
                qtj[j][d][64:128, :],
                start=True,
                stop=True,
            )
            drain_pending(sw)
            ex = expp.tile([128, 2 * JBLK], BF, tag="ex", name="ex")
            nc.scalar.activation(ex[:], sc[:], Exp)
            sw.pending.append((m, ex))

        av_free_est = [0.0]

        def drain_pending(sw):
            """Emit one deferred AV unless a new segment would have to wait
            for the av pair to drain (let exps run ahead instead). Hard cap:
            an exp's ex-tile allocation WARs against the AV 10 allocations
            back (expp ring) — if AVs lag more than ring-2, the sem graph
            deadlocks (exp waits AV, AV behind a blocked sc, sc waits exp)."""
            if len(sw.pending) <= PENDING_DEPTH:
                return
            if (
                len(sw.pending) < 7
                and pe_t < av_free_est[0]
                and sw.av is None
            ):
                return
            emit_pending_av(sw, stop=False)

        def close_segment(sw):
            """Evict/accumulate the open segment (or finish the sweep)."""
            final = sw.next_m == SM and bool(sw.pending)
            while sw.pending:
                emit_pending_av(sw, stop=len(sw.pending) == 1)
            av_free_est[0] = pe_t + CLOSE_LATENCY
            if sw.av is None:
                return
            av0, av1 = sw.av
            sw.av = None
            j, d = sw.j, sw.d
            if final and sw.pacc is None:
                # single full segment: normalize straight from psum (DVE)
                post = all(s.done for s in sweeps if s is not sw)
                norm(j, d, (av0[:, 0:260], av1[:, 0:260]), sbuf_src=False,
                     post_exp=post)
                sw.done = True
                return
            if sw.pacc is None:
                sw.pacc = paccp.tile(
                    [128, 520], F32, tag=f"pacc{j}{d}", name=f"pacc{j}{d}"
                )
                nc.vector.tensor_copy(sw.pacc[:, 0:260], av0[:, 0:260])
                nc.vector.tensor_copy(sw.pacc[:, 260:520], av1[:, 0:260])
            else:
                nc.vector.tensor_add(sw.pacc[:, 0:260], sw.pacc[:, 0:260], av0[:, 0:260])
                nc.vector.tensor_add(sw.pacc[:, 260:520], sw.pacc[:, 260:520], av1[:, 0:260])
            if final:
                post = all(s.done for s in sweeps if s is not sw)
                norm(j, d, (sw.pacc[:, 0:260], sw.pacc[:, 260:520]), sbuf_src=True,
                     post_exp=post)
                sw.done = True

        def norm(j, d, halves, sbuf_src=False, post_exp=False):
            if j not in att_state:
                att_state[j] = [
                    attp.tile([128, JBLK], BF, tag=f"att{qc}", name=f"att{qc}")
                    for qc in range(4)
                ]
            att_j = att_state[j]
            # Pool can't read PSUM; only pacc (SBUF) sources may use it
            mul_eng = nc.gpsimd if sbuf_src else nc.vector
            Copy = mybir.ActivationFunctionType.Copy
            for hh in range(2):
                src = halves[hh]
                h2 = 2 * d + hh
                for qc in range(4):
                    s0 = qc * 65
                    rcp = rcpp.tile([128, 1], F32, tag="rcp", name="rcp")
                    nc.vector.reciprocal(rcp[:], src[:, s0 + 64 : s0 + 65])
                    dst = att_j[qc][:, h2 * 64 : (h2 + 1) * 64]
                    mul_eng.tensor_scalar_mul(dst, src[:, s0 : s0 + 64], rcp[:])

        # ---- aux pieces (attT / wo) ----
        def attT_piece(j, pair, pool=None):
            pool = pool or prodp
            tag = "prod" if pool is prodp else "sc"
            att_j = att_state[j]
            attT_j = attT_state.setdefault(j, [None] * MD)
            tps = pool.tile([128, 1024], BF, tag=tag, name="atps")
            for half in range(2):
                kk = pair * 2 + half
                for qc in range(4):
                    nc.tensor.transpose(
                        tps[:, half * 512 + qc * 128 : half * 512 + (qc + 1) * 128],
                        att_j[qc][:, kk * 128 : (kk + 1) * 128],
                        identb_t[:],
                    )
            for half in range(2):
                kk = pair * 2 + half
                t = attTp.tile([128, JBLK], BF, tag=f"attT{kk}", name="attT")
                nc.vector.tensor_copy(t[:], tps[:, half * 512 : (half + 1) * 512])
                attT_j[kk] = t

        def wo_half(j, qc, pair):
            """Head-pairs 2*pair..2*pair+1 contribution to out[pair] rows."""
            attT_j = attT_state[j]
            wops = [
                prodp.tile([128, 512], F32, tag="prod", name=f"wops{nb}")
                for nb in range(2)
            ]
            for nb in range(2):
                for kk in (2 * pair, 2 * pair + 1):
                    nc.tensor.matmul(
                        wops[nb][:],
                        attT_j[kk][:, qc * 128 : (qc + 1) * 128],
                        wo_t[:, kk * D + nb * 512 : kk * D + nb * 512 + 512],
                        start=(kk == 2 * pair),
                        stop=(kk == 2 * pair + 1),
                    )
            ot = outp.tile([128, D], F32, tag="ot", name="ot")
            mrow = j * JBLK + qc * 128
            if all(s.done for s in sweeps):
                # tail: chain each half's DMA behind its eviction
                for nb in range(2):
                    nc.vector.tensor_copy(ot[:, nb * 512 : (nb + 1) * 512], wops[nb][:])
                    nc.sync.dma_start(
                        out=out[pair, mrow : mrow + 128, nb * 512 : (nb + 1) * 512],
                        in_=ot[:, nb * 512 : (nb + 1) * 512],
                    )
            else:
                nc.vector.tensor_copy(ot[:, 0:512], wops[0][:])
                nc.vector.tensor_copy(ot[:, 512:1024], wops[1][:])
                nc.sync.dma_start(out=out[pair, mrow : mrow + 128, :], in_=ot[:])

        # ---- build-time scheduler ----
        # production queue: (kind, args, pe_cost)
        prod_q = deque()
        prod_q.append(("qp", (0, 0), EST_PROJ))
        prod_q.append(("k", (0,), EST_KPASS))
        prod_q.append(("qp", (0, 1), EST_PROJ))
        prod_q.append(("k", (1,), EST_KPASS + 1100.0))
        prod_q.append(("qp", (0, 2), EST_PROJ))
        prod_q.append(("qp", (0, 3), EST_PROJ))
        prod_q.append(("qt", (0,), 900.0))
        prod_q.append(("k", (2,), EST_KPASS))
        prod_q.append(("v", (0,), EST_VPASS))
        prod_q.append(("k", (3,), EST_KPASS + 1100.0))
        prod_q.append(("v", (1,), EST_VPASS))
        prod_q.append(("k", (4,), EST_KPASS))
        prod_q.append(("k", (5,), EST_KPASS + 1100.0))
        for m in range(2, SM):
            if m + 6 < SM:
                prod_q.append(("hdma", (m + 6,), 0.0))
            if m in (2, 3):
                kcost = EST_KPASS + (1100.0 if (m + 4) % TG == TG - 1 else 0.0)
                prod_q.append(("k", (m + 4,), kcost))
            if m >= 8:
                # k staggered ahead so kt groups land early
                kcost = EST_KPASS + (1100.0 if m % TG == TG - 1 else 0.0)
                prod_q.append(("k", (m,), kcost))
            if 3 <= m <= 6:
                prod_q.append(("qdma", (1, m - 3), 0.0))
            if 5 <= m <= 8:
                prod_q.append(("qp", (1, m - 5), EST_PROJ))
            if 7 <= m <= 10:
                prod_q.append(("qdma", (2, m - 7), 0.0))
            if 9 <= m <= 12:
                prod_q.append(("qp", (2, m - 9), EST_PROJ))
            prod_q.append(("v", (m,), EST_VPASS))
            if m == 8:
                prod_q.append(("qt", (1,), 900.0))
            if m == 12:
                prod_q.append(("qt", (2,), 900.0))
        for j in (3,):
            prod_q.append(("qdma", (j, 0), 0.0))
            prod_q.append(("qdma", (j, 1), 0.0))
            for k in range(4):
                if k + 2 < 4:
                    prod_q.append(("qdma", (j, k + 2), 0.0))
                prod_q.append(("qp", (j, k), EST_PROJ))
            prod_q.append(("qt", (j,), 900.0))

        produced_k = set()
        produced_v = set()
        qt_ready = set()

        def run_prod(item):
            kind, args, _ = item
            if kind == "k":
                k_pass(*args)
                m = args[0]
                if m % TG == TG - 1:
                    for mm in range(m - TG + 1, m + 1):
                        produced_k.add(mm)
            elif kind == "v":
                v_pass(*args)
                produced_v.add(args[0])
            elif kind == "qp":
                q_pass(*args)
            elif kind == "qt":
                q_trans(*args)
                qt_ready.add(args[0])
            elif kind == "hdma":
                dma_h(*args)
            elif kind == "qdma":
                j, k = args
                q_dma(j, k)

        sweeps = [Sweep(j, d) for j in range(NJ) for d in range(MD)]
        aux_q = deque()  # (kind, args, pe_cost), dependency-ready aux work
        norm_count = {}  # j -> number of normed sweeps

        def on_sweep_done(sw):
            j = sw.j
            norm_count[j] = norm_count.get(j, 0) + 1
            dlist = [s.d for s in sweeps if s.j == j and s.done]
            for pair in range(2):
                if (
                    2 * pair in dlist
                    and 2 * pair + 1 in dlist
                    and (f"attT{pair}", j) not in emitted_aux
                ):
                    emitted_aux.add((f"attT{pair}", j))
                    # last attT of the kernel: sc ring is idle, borrow it
                    borrow_sc = j == NJ - 1 and pair == 1
                    aux_q.append(("attT", (j, pair, borrow_sc), 500.0))

        emitted_aux = set()

        def run_aux(item):
            kind, args, _ = item
            if kind == "attT":
                j, pair, borrow_sc = args
                attT_piece(j, pair, pool=scp if borrow_sc else prodp)
                for qc in range(4):
                    aux_q.append(("woh", (j, qc, pair), 950.0))
            elif kind == "woh":
                wo_half(*args)

        def sweep_ready(sw):
            return (
                not sw.done
                and sw.next_m < SM
                and sw.j in qt_ready
                and sw.next_m in produced_k
                and sw.next_m in produced_v
            )

        pe_t = 0.0
        act_t = 0.0
        active = None

        def emit_one_iter(sw):
            nonlocal pe_t, act_t, active
            if active is not None and active is not sw:
                close_segment(active)
                pe_t += 100.0
            active = sw
            emit_iter(sw)
            pe_t += EST_ITER_PE
            act_t = max(act_t + EST_ITER_ACT, pe_t + EST_SC + EST_ITER_ACT)

        def pick_sweep():
            if active is not None and sweep_ready(active):
                return active
            ready = [sw for sw in sweeps if sweep_ready(sw)]
            if not ready:
                return None

            def avail(sw):
                hi = sw.next_m
                while hi < SM and hi in produced_k and hi in produced_v:
                    hi += 1
                return hi - sw.next_m

            # finish low-j blocks first (staggers attT/wo aux); among same j
            # prefer the sweep with most available chunks (fewer switches)
            return min(ready, key=lambda sw: (sw.j, -avail(sw)))

        while True:
            remaining = [sw for sw in sweeps if not sw.done]
            if not remaining and not prod_q and not aux_q:
                break
            backlog = act_t - pe_t
            cand = pick_sweep() if backlog < BACKLOG_LO else None
            if cand is None and not prod_q and not aux_q:
                cand = pick_sweep()  # ACT-bound tail: keep sweeping
            if cand is not None:
                emit_one_iter(cand)
                if cand.next_m == SM:
                    close_segment(cand)
                    if cand is active:
                        active = None
                    on_sweep_done(cand)
                continue
            if prod_q:
                item = prod_q.popleft()
                run_prod(item)
                pe_t += item[2]
                continue
            if aux_q:
                item = aux_q.popleft()
                run_aux(item)
                pe_t += item[2]
                continue
            raise RuntimeError(
                f"scheduler deadlock: {[(sw.j, sw.d, sw.next_m) for sw in remaining]}"
            )
        assert all(sw.done for sw in sweeps)


def _wtile(w, dtype):
    """[K*128, N] -> [128, K*N] with tile[p, kk*N+c] = w[128*kk+p, c]."""
    kchunks = w.shape[0] // 128
    return np.ascontiguousarray(
        w.reshape(kchunks, 128, w.shape[1])
        .transpose(1, 0, 2)
        .reshape(128, kchunks * w.shape[1])
        .astype(dtype)
    )


def prep_core_inputs(positions, hidden_states, wq, wk, wv, wo):
    """Host-side sharding/pre-tiling. Returns list of 8 in_maps."""
    import ml_dtypes

    bf16 = ml_dtypes.bfloat16
    pos = np.asarray(positions).astype(np.float32)
    inv_freq = 1.0 / (ROPE_THETA ** (np.arange(0, DH, 2, dtype=np.float32) / DH))
    ang = pos[:, None] * inv_freq[None, :]  # [S, 32]
    cos_half = np.cos(ang).astype(np.float32)
    sin_half = np.sin(ang).astype(np.float32)
    cos_full = np.concatenate([cos_half, cos_half], axis=1)  # [S, 64]
    sin_signed = np.concatenate([-sin_half, sin_half], axis=1)  # [S, 64]
    cos_tiled = np.ascontiguousarray(
        cos_full.reshape(SM, 128, DH).transpose(1, 0, 2).reshape(128, SM * DH)
    )
    sin_tiled = np.ascontiguousarray(
        sin_signed.reshape(SM, 128, DH).transpose(1, 0, 2).reshape(128, SM * DH)
    )
    identb = np.eye(128, dtype=np.float32).astype(bf16)

    hs = np.asarray(hidden_states, dtype=np.float32)
    # hT[m, p, kk*128+cc] = hidden[b, 128m+cc, 128kk+p]
    hT_all = []
    for b in range(B):
        t = hs[b].reshape(SM, 128, KD, 128).transpose(0, 3, 2, 1)  # (m, p, kk, cc)
        hT_all.append(np.ascontiguousarray(t.reshape(SM, 128, D)).astype(bf16))

    in_maps = []
    for c in range(NCORES):
        b = c // 2
        g = c % 2
        sl = slice(g * MC, (g + 1) * MC)
        in_maps.append(
            {
                "hT": hT_all[b],
                "wq": _wtile(np.asarray(wq, np.float32)[:, sl], bf16),
                "wk": _wtile(np.asarray(wk, np.float32)[:, sl], bf16),
                "wv": _wtile(np.asarray(wv, np.float32)[:, sl], bf16),
                "wo": _wtile(np.asarray(wo, np.float32)[sl, :], bf16),
                "cosq": cos_tiled,
                "sinq": sin_tiled,
                "identb": identb,
            }
        )
    return in_maps


_NC_CACHE = {}


def get_nc():
    if "nc" not in _NC_CACHE:
        _NC_CACHE["nc"] = build_nc()
    return _NC_CACHE["nc"]


def kernel(positions, hidden_states, wq, wk, wv, wo):
    in_dtype = np.asarray(hidden_states).dtype
    in_maps = prep_core_inputs(positions, hidden_states, wq, wk, wv, wo)
    nc = get_nc()
    res = bass_utils.run_bass_kernel_spmd(nc, in_maps, core_ids=list(range(NCORES)))
    outs = np.empty((B, S, D), dtype=np.float32)
    for b in range(B):
        o0 = res.results[2 * b]["out"]
        o1 = res.results[2 * b + 1]["out"]
        outs[b] = (o0[0] + o0[1]) + (o1[0] + o1[1])
    return outs.astype(in_dtype, copy=False)


# revision 105
# speedup vs baseline: 1.1908x; 1.0000x over previous
"""Trainium2 Bass kernel for Chronos2Attention (B=4, S=2048, D=1024, H=16, Dh=64).

Sharding: 8 cores = 4 batches x 2 head-groups. Core c handles batch c//2 and
heads 8*(c%2) .. 8*(c%2)+7 (wq/wk/wv column-sharded, wo row-sharded); host sums
the two partial [S, D] outputs per batch at gather time.

v4 design (vs v3): fully fused production/attention pipeline.
  - All PE-side tensors bf16 (h, weights, roped q/k, kt, v, attn) -> every
    matmul/transpose runs at 1 cyc/row; halved DMA.
  - K/V/Q production streams through a 2-buf [128,512]f32 PSUM "prod" pool
    just-in-time; attention sweeps consume chunks as they appear. j0's Q
    projections borrow the (still idle) av banks so K can start in parallel;
    head k-ropes run on Pool to keep the DVE queue short.
  - Sweeps (j = 512-query block, d = head pair) are SEGMENTED: AV accumulates
    in a single PSUM bank-pair per segment (2 heads x 4qc x 65 cols with the
    ones/denominator column), partial segments accumulated into SBUF f32
    "pacc" tiles so 2 av banks suffice while many sweeps are in flight
    against a limited set of produced chunks.
  - AV matmuls are emitted LAZILY (deque per sweep, depth-capped by the
    ex-tile ring) so exps can run ahead across av-bank handoffs; the stop
    flag lands on the true last matmul of each segment.
  - Emission is availability-driven: a tiny cost model interleaves production
    passes, sweep iterations (sc->exp->AV), and aux work (norm/attT/wo/q-proj)
    to keep ACT (exp, the ~266us floor) saturated and PE dense. Engines:
    ACT = exp only (anything else deadlock-prone via ring WARs); DVE = ropes,
    psum evictions, pacc, norms-from-psum; Pool = head ropes + norms-from-pacc.
  - Wo is computed in head-pair halves into out[2, S, D] (host sums 4 partial
    outputs per batch) so each j block's Wo work drains early; the final
    block's last attT borrows the then-idle sc ring.
  - PSUM budget: sc ring 2x[128,1024] (4 banks) + av 2x[128,512] (2) +
    prod 2x[128,512] (2) = 8 banks.
"""

from collections import deque

import numpy as np

import concourse.bacc as bacc
import concourse.mybir as mybir
import concourse.tile as tile
from concourse import bass_utils

# Problem shapes (hardcoded per spec)
B = 4
S = 2048
D = 1024
H = 16
DH = 64
ROPE_THETA = 10000.0
NCORES = 8
HC = H // 2  # heads per core
MC = HC * DH  # 512, per-core projection width

SM = S // 128  # 16 seq chunks
KD = D // 128  # 8 contraction chunks for projections
MD = HC // 2  # 4 head-pair sweeps per j block
JBLK = 512
NJ = S // JBLK  # 4
TG = 2  # k-transpose group size

F32 = mybir.dt.float32
BF = mybir.dt.bfloat16

# emission cost estimates (ns) for the build-time scheduler
EST_PROJ = 1800.0
EST_KPASS = 2800.0  # proj + rope serialization through the prod ring
EST_VPASS = 2200.0
EST_ITER_PE = 700.0
EST_ITER_ACT = 1045.0
EST_SC = 450.0
BACKLOG_LO = 6000.0  # emit sweep iters while ACT backlog below this


def build_nc():
    nc = bacc.Bacc("TRN2", target_bir_lowering=False, debug=False, num_devices=1)

    hT = nc.dram_tensor("hT", [SM, 128, D], BF, kind="ExternalInput").ap()
    wq = nc.dram_tensor("wq", [128, KD * MC], BF, kind="ExternalInput").ap()
    wk = nc.dram_tensor("wk", [128, KD * MC], BF, kind="ExternalInput").ap()
    wv = nc.dram_tensor("wv", [128, KD * MC], BF, kind="ExternalInput").ap()
    wo = nc.dram_tensor("wo", [128, MD * D], BF, kind="ExternalInput").ap()
    cosq = nc.dram_tensor("cosq", [128, SM * DH], F32, kind="ExternalInput").ap()
    sinq = nc.dram_tensor("sinq", [128, SM * DH], F32, kind="ExternalInput").ap()
    identb = nc.dram_tensor("identb", [128, 128], BF, kind="ExternalInput").ap()
    # out[ph]: contribution of head-pairs 2ph..2ph+1 (summed host-side)
    out = nc.dram_tensor("out", [2, S, D], F32, kind="ExternalOutput").ap()

    with tile.TileContext(nc) as tc:
        _build_body(nc, tc, hT, wq, wk, wv, wo, cosq, sinq, identb, out)
    nc.compile()
    return nc


def _build_body(nc, tc, hT, wq, wk, wv, wo, cosq, sinq, identb, out):
    from contextlib import ExitStack

    Exp = mybir.ActivationFunctionType.Exp

    with ExitStack() as ctx:
        # ---- persistent SBUF ----
        persist = ctx.enter_context(tc.tile_pool(name="persist", bufs=1))
        kt = [persist.tile([128, S], BF, tag=f"kt{d}", name=f"kt{d}") for d in range(MD)]
        # v1[p, m*520 + h*65 + e]: e<64 -> V dims, e==64 -> ones (softmax denom)
        v1 = persist.tile([128, SM * (HC * 65)], BF, tag="v1", name="v1")
        wq_t = persist.tile([128, KD * MC], BF, tag="w_q", name="w_q")
        wk_t = persist.tile([128, KD * MC], BF, tag="w_k", name="w_k")
        wv_t = persist.tile([128, KD * MC], BF, tag="w_v", name="w_v")
        wo_t = persist.tile([128, MD * D], BF, tag="wo", name="wo_t")
        cos_t = persist.tile([128, SM * DH], F32, tag="cos", name="cos_t")
        sin_t = persist.tile([128, SM * DH], F32, tag="sin", name="sin_t")
        identb_t = persist.tile([128, 128], BF, tag="identb", name="identb_t")

        # ---- working SBUF pools ----
        hpool = ctx.enter_context(tc.tile_pool(name="hprod", bufs=8))
        hqpool = ctx.enter_context(tc.tile_pool(name="hq", bufs=4))
        rkpool = ctx.enter_context(tc.tile_pool(name="rk", bufs=4))
        kfpool = ctx.enter_context(tc.tile_pool(name="kf", bufs=2))
        rqpool = ctx.enter_context(tc.tile_pool(name="rq", bufs=4))
        qpool = ctx.enter_context(tc.tile_pool(name="qtj", bufs=3))
        tmpp = ctx.enter_context(tc.tile_pool(name="ropetmp", bufs=3))
        expp = ctx.enter_context(tc.tile_pool(name="expp", bufs=9))
        paccp = ctx.enter_context(tc.tile_pool(name="pacc", bufs=1))
        rcpp = ctx.enter_context(tc.tile_pool(name="rcpp", bufs=8))
        attp = ctx.enter_context(tc.tile_pool(name="attp", bufs=3))
        attTp = ctx.enter_context(tc.tile_pool(name="attTp", bufs=3))
        outp = ctx.enter_context(tc.tile_pool(name="outp", bufs=2))

        # ---- PSUM: 4 + 2 + 2 = 8 banks ----
        scp = ctx.enter_context(tc.tile_pool(name="scp", bufs=2, space="PSUM"))
        avp = ctx.enter_context(tc.tile_pool(name="avp", bufs=2, space="PSUM"))
        prodp = ctx.enter_context(tc.tile_pool(name="prodp", bufs=2, space="PSUM"))

        # ---- init: DMAs ordered for earliest first sweep ----
        h_tiles = {}

        def dma_h(m):
            t = hpool.tile([128, D], BF, tag="h", name="h_m")
            nc.sync.dma_start(out=t[:], in_=hT[m])
            h_tiles[m] = t

        half = 4 * MC
        csh = SM * DH // 2
        csq = 4 * DH  # chunks 0-3: enough for j0's q-ropes and k0-k3
        nc.sync.dma_start(out=wq_t[:, 0:half], in_=wq[:, 0:half])
        nc.sync.dma_start(out=cos_t[:, 0:csq], in_=cosq[:, 0:csq])
        nc.sync.dma_start(out=sin_t[:, 0:csq], in_=sinq[:, 0:csq])
        dma_h(0)
        dma_h(1)
        nc.sync.dma_start(out=wq_t[:, half:], in_=wq[:, half:])
        nc.sync.dma_start(out=wk_t[:, 0:half], in_=wk[:, 0:half])
        nc.sync.dma_start(out=wk_t[:, half:], in_=wk[:, half:])
        nc.sync.dma_start(out=cos_t[:, csq:csh], in_=cosq[:, csq:csh])
        nc.sync.dma_start(out=sin_t[:, csq:csh], in_=sinq[:, csq:csh])
        nc.sync.dma_start(out=wv_t[:, 0:half], in_=wv[:, 0:half])
        nc.sync.dma_start(out=identb_t[:], in_=identb)
        dma_h(2)
        dma_h(3)
        nc.sync.dma_start(out=cos_t[:, csh:], in_=cosq[:, csh:])
        nc.sync.dma_start(out=sin_t[:, csh:], in_=sinq[:, csh:])
        nc.sync.dma_start(out=wv_t[:, half:], in_=wv[:, half:])
        dma_h(4)
        dma_h(5)
        dma_h(6)
        dma_h(7)
        nc.sync.dma_start(out=wo_t[:], in_=wo)

        # PE warmup (p-state ramp) on dep-free memset tiles
        wu = persist.tile([128, 512], F32, tag="wu", name="wu")
        nc.vector.memset(wu[:], 0.0)
        wups = prodp.tile([128, 512], F32, tag="prod", name="wups")
        for i in range(2):
            nc.tensor.matmul(
                wups[:], wu[:, 0:128], wu[:], start=(i == 0), stop=(i == 1)
            )

        # ones columns of v1 (Pool memset + DVE broadcast); warm the exp table
        oneb = persist.tile([128, 1], BF, tag="oneb", name="oneb")
        nc.gpsimd.memset(oneb[:], 1.0)
        nc.vector.tensor_copy(
            v1[:].rearrange("p (m h e) -> p m h e", m=SM, h=HC)[:, :, :, 64:65],
            oneb[:, None, None, 0:1].broadcast_to([128, SM, HC, 1]),
        )
        warm = persist.tile([1, 16], F32, tag="warm", name="warm")
        nc.vector.memset(warm[:], 0.0)
        nc.scalar.activation(warm[:], warm[:], Exp)

        # ---- shared helpers ----
        def rope(src, r, m, eng):
            """Natural-layout RoPE: src [128 seq, HC*DH] -> r bf16.
            eng=nc.vector reads PSUM directly (low latency, K path);
            eng=nc.gpsimd needs an SBUF source (Q path, off critical path)."""
            cos_m = cos_t[:, None, m * DH : (m + 1) * DH]
            sin_m = sin_t[:, m * DH : (m + 1) * DH]
            tc_ = tmpp.tile([128, MC], F32, tag="tc", name="tc_")
            ts_ = tmpp.tile([128, MC], F32, tag="ts", name="ts_")
            p3 = src.rearrange("p (h e) -> p h e", h=HC)
            t3 = ts_[:].rearrange("p (h e) -> p h e", h=HC)
            eng.tensor_mul(
                tc_[:].rearrange("p (h e) -> p h e", h=HC),
                p3,
                cos_m.broadcast_to([128, HC, DH]),
            )
            eng.tensor_mul(
                t3[:, :, 0:32],
                p3[:, :, 32:64],
                sin_m[:, None, 0:32].broadcast_to([128, HC, 32]),
            )
            eng.tensor_mul(
                t3[:, :, 32:64],
                p3[:, :, 0:32],
                sin_m[:, None, 32:64].broadcast_to([128, HC, 32]),
            )
            eng.tensor_add(r[:], tc_[:], ts_[:])

        def proj(h_m, w_t, pool=None, tag="prod"):
            ps = (pool or prodp).tile([128, MC], F32, tag=tag, name="ps")
            for kk in range(KD):
                nc.tensor.matmul(
                    ps[:],
                    h_m[:, kk * 128 : (kk + 1) * 128],
                    w_t[:, kk * MC : (kk + 1) * MC],
                    start=(kk == 0),
                    stop=(kk == KD - 1),
                )
            return ps

        # ---- production pieces ----
        rot_k = [None] * TG
        qrot = {j: [None] * 4 for j in range(NJ)}
        qtj = {}

        def k_pass(m):
            ps = proj(h_tiles[m], wk_t)
            r = rkpool.tile([128, MC], BF, tag="rk", name="rk")
            if m < 2:
                # head: keep the DVE queue free for q-ropes; rope on Pool
                kf = kfpool.tile([128, MC], F32, tag="kf", name="kf")
                nc.vector.tensor_copy(kf[:], ps[:])
                rope(kf[:], r, m, nc.gpsimd)
            else:
                rope(ps[:], r, m, nc.vector)
            rot_k[m % TG] = r
            if m % TG == TG - 1:
                m0 = m - (TG - 1)
                W = TG * 128
                for dpair in range(2):
                    tps = prodp.tile([128, 2 * W], BF, tag="prod", name="ktps")
                    for half in range(2):
                        d = dpair * 2 + half
                        for mm in range(TG):
                            nc.tensor.transpose(
                                tps[:, half * W + mm * 128 : half * W + (mm + 1) * 128],
                                rot_k[mm][:, d * 128 : (d + 1) * 128],
                                identb_t[:],
                            )
                    for half in range(2):
                        d = dpair * 2 + half
                        nc.vector.tensor_copy(
                            kt[d][:, m0 * 128 : m0 * 128 + W],
                            tps[:, half * W : (half + 1) * W],
                        )

        def v_pass(m):
            ps = proj(h_tiles[m], wv_t)
            dst = v1[:, m * (HC * 65) : (m + 1) * (HC * 65)].rearrange(
                "p (h e) -> p h e", h=HC
            )[:, :, 0:64]
            nc.vector.tensor_copy(dst, ps[:].rearrange("p (h e) -> p h e", h=HC))
            h_tiles.pop(m)

        def q_pass(j, k):
            m = j * 4 + k
            if j == 0:
                h_m = h_tiles[m]
            else:
                h_m = h_tiles.pop(("q", m))
            # j0's projections borrow the av banks (idle until the first AV)
            # so K production streams through prodp in parallel
            ps = proj(h_m, wq_t, pool=avp if j == 0 else None,
                      tag="av" if j == 0 else "prod")
            r = rqpool.tile([128, MC], BF, tag="rq", name="rq")
            rope(ps[:], r, m, nc.vector)
            qrot[j][k] = r

        def q_dma(j, k):
            m = j * 4 + k
            t = hqpool.tile([128, D], BF, tag="hq", name="hq")
            nc.sync.dma_start(out=t[:], in_=hT[m])
            h_tiles[("q", m)] = t

        def q_trans(j):
            tiles = [
                qpool.tile([128, JBLK], BF, tag=f"qt{d}", name=f"qt{d}")
                for d in range(MD)
            ]
            for dpair in range(2):
                tps = prodp.tile([128, 1024], BF, tag="prod", name="qtps")
                for half in range(2):
                    d = dpair * 2 + half
                    for mm in range(4):
                        nc.tensor.transpose(
                            tps[:, half * 512 + mm * 128 : half * 512 + (mm + 1) * 128],
                            qrot[j][mm][:, d * 128 : (d + 1) * 128],
                            identb_t[:],
                        )
                for half in range(2):
                    d = dpair * 2 + half
                    nc.vector.tensor_copy(
                        tiles[d][:], tps[:, half * 512 : (half + 1) * 512]
                    )
            qrot[j] = [None] * 4
            qtj[j] = tiles

        # ---- sweep machinery ----
        # state per sweep (j,d): next_m, av pair or None, pacc tile, pending AV
        att_state = {}  # j -> [att tiles per qc]
        attT_state = {}  # j -> [attT tiles per kk]

        PENDING_DEPTH = 2  # min AV emission lag behind exp (iters)
        CLOSE_LATENCY = 2600.0  # est ns from close emission to av-bank free

        class Sweep:
            def __init__(self, j, d):
                self.j = j
                self.d = d
                self.next_m = 0
                self.av = None  # (av0, av1) psum pair while segment open
                self.seg_start = True  # next pending AV opens a segment
                self.pacc = None
                self.pending = deque()  # (m, ex) awaiting AV emission
                self.done = False

        def emit_pending_av(sw, stop):
            m, ex = sw.pending.popleft()
            if sw.av is None:
                sw.av = (
                    avp.tile([128, 512], F32, tag="av", name="av0"),
                    avp.tile([128, 512], F32, tag="av", name="av1"),
                )
                sw.seg_start = True
            for hh in range(2):
                h2 = 2 * sw.d + hh
                vs = m * (HC * 65) + h2 * 65
                for qc in range(4):
                    nc.tensor.matmul(
                        sw.av[hh][:, qc * 65 : (qc + 1) * 65],
                        ex[:, hh * JBLK + qc * 128 : hh * JBLK + (qc + 1) * 128],
                        v1[:, vs : vs + 65],
                        start=(sw.seg_start and qc == 0),
                        stop=(stop and qc == 3),
                    )
            sw.seg_start = False

        def emit_iter(sw):
            """One m iteration: sc matmuls + exp; AV of previous iter."""
            m = sw.next_m
            sw.next_m += 1
            d, j = sw.d, sw.j
            sc = scp.tile([128, 2 * JBLK], F32, tag="sc", name="sc")
            nc.tensor.matmul(
                sc[:, 0:JBLK],
                kt[d][0:64, m * 128 : (m + 1) * 128],
                qtj[j][d][0:64, :],
                start=True,
                stop=True,
            )
            nc.tensor.matmul(
                sc[:, JBLK : 2 * JBLK],
                kt[d][64:128, m * 128 : (m + 1) * 128],
                qtj[j][d][64:128, :],
                start=True,
                stop=True,
            )
            drain_pending(sw)
            ex = expp.tile([128, 2 * JBLK], BF, tag="ex", name="ex")
            nc.scalar.activation(ex[:], sc[:], Exp)
            sw.pending.append((m, ex))

        av_free_est = [0.0]

        def drain_pending(sw):
            """Emit one deferred AV unless a new segment would have to wait
            for the av pair to drain (let exps run ahead instead). Hard cap:
            an exp's ex-tile allocation WARs against the AV 10 allocations
            back (expp ring) — if AVs lag more than ring-2, the sem graph
            deadlocks (exp waits AV, AV behind a blocked sc, sc waits exp)."""
            if len(sw.pending) <= PENDING_DEPTH:
                return
            if (
                len(sw.pending) < 7
                and pe_t < av_free_est[0]
                and sw.av is None
            ):
                return
            emit_pending_av(sw, stop=False)

        def close_segment(sw):
            """Evict/accumulate the open segment (or finish the sweep)."""
            final = sw.next_m == SM and bool(sw.pending)
            while sw.pending:
                emit_pending_av(sw, stop=len(sw.pending) == 1)
            av_free_est[0] = pe_t + CLOSE_LATENCY
            if sw.av is None:
                return
            av0, av1 = sw.av
            sw.av = None
            j, d = sw.j, sw.d
            if final and sw.pacc is None:
                # single full segment: normalize straight from psum (DVE)
                post = all(s.done for s in sweeps if s is not sw)
                norm(j, d, (av0[:, 0:260], av1[:, 0:260]), sbuf_src=False,
                     post_exp=post)
                sw.done = True
                return
            if sw.pacc is None:
                sw.pacc = paccp.tile(
                    [128, 520], F32, tag=f"pacc{j}{d}", name=f"pacc{j}{d}"
                )
                nc.vector.tensor_copy(sw.pacc[:, 0:260], av0[:, 0:260])
                nc.vector.tensor_copy(sw.pacc[:, 260:520], av1[:, 0:260])
            else:
                nc.vector.tensor_add(sw.pacc[:, 0:260], sw.pacc[:, 0:260], av0[:, 0:260])
                nc.vector.tensor_add(sw.pacc[:, 260:520], sw.pacc[:, 260:520], av1[:, 0:260])
            if final:
                post = all(s.done for s in sweeps if s is not sw)
                norm(j, d, (sw.pacc[:, 0:260], sw.pacc[:, 260:520]), sbuf_src=True,
                     post_exp=post)
                sw.done = True

        def norm(j, d, halves, sbuf_src=False, post_exp=False):
            if j not in att_state:
                att_state[j] = [
                    attp.tile([128, JBLK], BF, tag=f"att{qc}", name=f"att{qc}")
                    for qc in range(4)
                ]
            att_j = att_state[j]
            # Pool can't read PSUM; only pacc (SBUF) sources may use it
            mul_eng = nc.gpsimd if sbuf_src else nc.vector
            Copy = mybir.ActivationFunctionType.Copy
            for hh in range(2):
                src = halves[hh]
                h2 = 2 * d + hh
                for qc in range(4):
                    s0 = qc * 65
                    rcp = rcpp.tile([128, 1], F32, tag="rcp", name="rcp")
                    nc.vector.reciprocal(rcp[:], src[:, s0 + 64 : s0 + 65])
                    dst = att_j[qc][:, h2 * 64 : (h2 + 1) * 64]
                    mul_eng.tensor_scalar_mul(dst, src[:, s0 : s0 + 64], rcp[:])

        # ---- aux pieces (attT / wo) ----
        def attT_piece(j, pair, pool=None):
            pool = pool or prodp
            tag = "prod" if pool is prodp else "sc"
            att_j = att_state[j]
            attT_j = attT_state.setdefault(j, [None] * MD)
            tps = pool.tile([128, 1024], BF, tag=tag, name="atps")
            for half in range(2):
                kk = pair * 2 + half
                for qc in range(4):
                    nc.tensor.transpose(
                        tps[:, half * 512 + qc * 128 : half * 512 + (qc + 1) * 128],
                        att_j[qc][:, kk * 128 : (kk + 1) * 128],
                        identb_t[:],
                    )
            for half in range(2):
                kk = pair * 2 + half
                t = attTp.tile([128, JBLK], BF, tag=f"attT{kk}", name="attT")
                nc.vector.tensor_copy(t[:], tps[:, half * 512 : (half + 1) * 512])
                attT_j[kk] = t

        def wo_half(j, qc, pair):
            """Head-pairs 2*pair..2*pair+1 contribution to out[pair] rows."""
            attT_j = attT_state[j]
            wops = [
                prodp.tile([128, 512], F32, tag="prod", name=f"wops{nb}")
                for nb in range(2)
            ]
            for nb in range(2):
                for kk in (2 * pair, 2 * pair + 1):
                    nc.tensor.matmul(
                        wops[nb][:],
                        attT_j[kk][:, qc * 128 : (qc + 1) * 128],
                        wo_t[:, kk * D + nb * 512 : kk * D + nb * 512 + 512],
                        start=(kk == 2 * pair),
                        stop=(kk == 2 * pair + 1),
                    )
            ot = outp.tile([128, D], F32, tag="ot", name="ot")
            mrow = j * JBLK + qc * 128
            if all(s.done for s in sweeps):
                # tail: chain each half's DMA behind its eviction
                for nb in range(2):
                    nc.vector.tensor_copy(ot[:, nb * 512 : (nb + 1) * 512], wops[nb][:])
                    nc.sync.dma_start(
                        out=out[pair, mrow : mrow + 128, nb * 512 : (nb + 1) * 512],
                        in_=ot[:, nb * 512 : (nb + 1) * 512],
                    )
            else:
                nc.vector.tensor_copy(ot[:, 0:512], wops[0][:])
                nc.vector.tensor_copy(ot[:, 512:1024], wops[1][:])
                nc.sync.dma_start(out=out[pair, mrow : mrow + 128, :], in_=ot[:])

        # ---- build-time scheduler ----
        # production queue: (kind, args, pe_cost)
        prod_q = deque()
        prod_q.append(("qp", (0, 0), EST_PROJ))
        prod_q.append(("k", (0,), EST_KPASS))
        prod_q.append(("qp", (0, 1), EST_PROJ))
        prod_q.append(("k", (1,), EST_KPASS + 1100.0))
        prod_q.append(("qp", (0, 2), EST_PROJ))
        prod_q.append(("qp", (0, 3), EST_PROJ))
        prod_q.append(("qt", (0,), 900.0))
        prod_q.append(("k", (2,), EST_KPASS))
        prod_q.append(("v", (0,), EST_VPASS))
        prod_q.append(("k", (3,), EST_KPASS + 1100.0))
        prod_q.append(("v", (1,), EST_VPASS))
        prod_q.append(("k", (4,), EST_KPASS))
        prod_q.append(("k", (5,), EST_KPASS + 1100.0))
        for m in range(2, SM):
            if m + 6 < SM:
                prod_q.append(("hdma", (m + 6,), 0.0))
            if m in (2, 3):
                kcost = EST_KPASS + (1100.0 if (m + 4) % TG == TG - 1 else 0.0)
                prod_q.append(("k", (m + 4,), kcost))
            if m >= 8:
                # k staggered ahead so kt groups land early
                kcost = EST_KPASS + (1100.0 if m % TG == TG - 1 else 0.0)
                prod_q.append(("k", (m,), kcost))
            if 3 <= m <= 6:
                prod_q.append(("qdma", (1, m - 3), 0.0))
            if 5 <= m <= 8:
                prod_q.append(("qp", (1, m - 5), EST_PROJ))
            if 7 <= m <= 10:
                prod_q.append(("qdma", (2, m - 7), 0.0))
            if 9 <= m <= 12:
                prod_q.append(("qp", (2, m - 9), EST_PROJ))
            prod_q.append(("v", (m,), EST_VPASS))
            if m == 8:
                prod_q.append(("qt", (1,), 900.0))
            if m == 12:
                prod_q.append(("qt", (2,), 900.0))
        for j in (3,):
            prod_q.append(("qdma", (j, 0), 0.0))
            prod_q.append(("qdma", (j, 1), 0.0))
            for k in range(4):
                if k + 2 < 4:
                    prod_q.append(("qdma", (j, k + 2), 0.0))
                prod_q.append(("qp", (j, k), EST_PROJ))
            prod_q.append(("qt", (j,), 900.0))

        produced_k = set()
        produced_v = set()
        qt_ready = set()

        def run_prod(item):
            kind, args, _ = item
            if kind == "k":
                k_pass(*args)
                m = args[0]
                if m % TG == TG - 1:
                    for mm in range(m - TG + 1, m + 1):
                        produced_k.add(mm)
            elif kind == "v":
                v_pass(*args)
                produced_v.add(args[0])
            elif kind == "qp":
                q_pass(*args)
            elif kind == "qt":
                q_trans(*args)
                qt_ready.add(args[0])
            elif kind == "hdma":
                dma_h(*args)
            elif kind == "qdma":
                j, k = args
                q_dma(j, k)

        sweeps = [Sweep(j, d) for j in range(NJ) for d in range(MD)]
        aux_q = deque()  # (kind, args, pe_cost), dependency-ready aux work
        norm_count = {}  # j -> number of normed sweeps

        def on_sweep_done(sw):
            j = sw.j
            norm_count[j] = norm_count.get(j, 0) + 1
            dlist = [s.d for s in sweeps if s.j == j and s.done]
            for pair in range(2):
                if (
                    2 * pair in dlist
                    and 2 * pair + 1 in dlist
                    and (f"attT{pair}", j) not in emitted_aux
                ):
                    emitted_aux.add((f"attT{pair}", j))
                    # last attT of the kernel: sc ring is idle, borrow it
                    borrow_sc = j == NJ - 1 and pair == 1
                    aux_q.append(("attT", (j, pair, borrow_sc), 500.0))

        emitted_aux = set()

        def run_aux(item):
            kind, args, _ = item
            if kind == "attT":
                j, pair, borrow_sc = args
                attT_piece(j, pair, pool=scp if borrow_sc else prodp)
                for qc in range(4):
                    aux_q.append(("woh", (j, qc, pair), 950.0))
            elif kind == "woh":
                wo_half(*args)

        def sweep_ready(sw):
            return (
                not sw.done
                and sw.next_m < SM
                and sw.j in qt_ready
                and sw.next_m in produced_k
                and sw.next_m in produced_v
            )

        pe_t = 0.0
        act_t = 0.0
        active = None

        def emit_one_iter(sw):
            nonlocal pe_t, act_t, active
            if active is not None and active is not sw:
                close_segment(active)
                pe_t += 100.0
            active = sw
            emit_iter(sw)
            pe_t += EST_ITER_PE
            act_t = max(act_t + EST_ITER_ACT, pe_t + EST_SC + EST_ITER_ACT)

        def pick_sweep():
            if active is not None and sweep_ready(active):
                return active
            ready = [sw for sw in sweeps if sweep_ready(sw)]
            if not ready:
                return None

            def avail(sw):
                hi = sw.next_m
                while hi < SM and hi in produced_k and hi in produced_v:
                    hi += 1
                return hi - sw.next_m

            # finish low-j blocks first (staggers attT/wo aux); among same j
            # prefer the sweep with most available chunks (fewer switches)
            return min(ready, key=lambda sw: (sw.j, -avail(sw)))

        while True:
            remaining = [sw for sw in sweeps if not sw.done]
            if not remaining and not prod_q and not aux_q:
                break
            backlog = act_t - pe_t
            cand = pick_sweep() if backlog < BACKLOG_LO else None
            if cand is None and not prod_q and not aux_q:
                cand = pick_sweep()  # ACT-bound tail: keep sweeping
            if cand is not None:
                emit_one_iter(cand)
                if cand.next_m == SM:
                    close_segment(cand)
                    if cand is active:
                        active = None
                    on_sweep_done(cand)
                continue
            if prod_q:
                item = prod_q.popleft()
                run_prod(item)
                pe_t += item[2]
                continue
            if aux_q:
                item = aux_q.popleft()
                run_aux(item)
                pe_t += item[2]
                continue
            raise RuntimeError(
                f"scheduler deadlock: {[(sw.j, sw.d, sw.next_m) for sw in remaining]}"
            )
        assert all(sw.done for sw in sweeps)


def _wtile(w, dtype):
    """[K*128, N] -> [128, K*N] with tile[p, kk*N+c] = w[128*kk+p, c]."""
    kchunks = w.shape[0] // 128
    return np.ascontiguousarray(
        w.reshape(kchunks, 128, w.shape[1])
        .transpose(1, 0, 2)
        .reshape(128, kchunks * w.shape[1])
        .astype(dtype)
    )


def prep_core_inputs(positions, hidden_states, wq, wk, wv, wo):
    """Host-side sharding/pre-tiling. Returns list of 8 in_maps."""
    import ml_dtypes

    bf16 = ml_dtypes.bfloat16
    pos = np.asarray(positions).astype(np.float32)
    inv_freq = 1.0 / (ROPE_THETA ** (np.arange(0, DH, 2, dtype=np.float32) / DH))
    ang = pos[:, None] * inv_freq[None, :]  # [S, 32]
    cos_half = np.cos(ang).astype(np.float32)
    sin_half = np.sin(ang).astype(np.float32)
    cos_full = np.concatenate([cos_half, cos_half], axis=1)  # [S, 64]
    sin_signed = np.concatenate([-sin_half, sin_half], axis=1)  # [S, 64]
    cos_tiled = np.ascontiguousarray(
        cos_full.reshape(SM, 128, DH).transpose(1, 0, 2).reshape(128, SM * DH)
    )
    sin_tiled = np.ascontiguousarray(
        sin_signed.reshape(SM, 128, DH).transpose(1, 0, 2).reshape(128, SM * DH)
    )
    identb = np.eye(128, dtype=np.float32).astype(bf16)

    hs = np.asarray(hidden_states, dtype=np.float32)
    # hT[m, p, kk*128+cc] = hidden[b, 128m+cc, 128kk+p]
    hT_all = []
    for b in range(B):
        t = hs[b].reshape(SM, 128, KD, 128).transpose(0, 3, 2, 1)  # (m, p, kk, cc)
        hT_all.append(np.ascontiguousarray(t.reshape(SM, 128, D)).astype(bf16))

    in_maps = []
    for c in range(NCORES):
        b = c // 2
        g = c % 2
        sl = slice(g * MC, (g + 1) * MC)
        in_maps.append(
            {
                "hT": hT_all[b],
                "wq": _wtile(np.asarray(wq, np.float32)[:, sl], bf16),
                "wk": _wtile(np.asarray(wk, np.float32)[:, sl], bf16),
                "wv": _wtile(np.asarray(wv, np.float32)[:, sl], bf16),
                "wo": _wtile(np.asarray(wo, np.float32)[sl, :], bf16),
                "cosq": cos_tiled,
                "sinq": sin_tiled,
                "identb": identb,
            }
        )
    return in_maps


_NC_CACHE = {}


def get_nc():
    if "nc" not in _NC_CACHE:
        _NC_CACHE["nc"] = build_nc()
    return _NC_CACHE["nc"]


def kernel(positions, hidden_states, wq, wk, wv, wo):
    in_dtype = np.asarray(hidden_states).dtype
    in_maps = prep_core_inputs(positions, hidden_states, wq, wk, wv, wo)
    nc = get_nc()
    res = bass_utils.run_bass_kernel_spmd(nc, in_maps, core_ids=list(range(NCORES)))
    outs = np.empty((B, S, D), dtype=np.float32)
    for b in range(B):
        o0 = res.results[2 * b]["out"]
        o1 = res.results[2 * b + 1]["out"]
        outs[b] = (o0[0] + o0[1]) + (o1[0] + o1[1])
    return outs.astype(in_dtype, copy=False)
